# revision 26
# baseline (speedup 1.0000x reference)
"""Grouped linear (MoE routing) kernel for 8 Trainium2 NeuronCores.

out[n] = x[n] @ weight[g[n]].T + bias[g[n]]

Strategy: expert-parallel. group_indices is (assumed) sorted; host code
computes per-group row ranges, pads each group's rows to a common
capacity C (multiple of 128), and core g computes the dense GEMM
  out_g = x_g @ weight[g].T + bias[g]
entirely on-core with no collectives. Host gathers/scatters rows.

Two per-core program modes (MODE below):

  bf16 — x and W cast to bf16; PE runs 1 col/cycle; fp32 PSUM + fp32
    bias/output. ~232 us/core PE floor at C=2176. (previous baseline)

  fp8 — error-compensated fp8 split using DoubleRow perf mode, which
    streams fp8e4m3 matmuls at 0.5 cycles/row with 2 k-subtiles folded
    per instruction (4x bf16 throughput per product). Host splits
      x*sx = x8 + r8   (r8 = fp8 of the quantization residual, same scale)
      W*sw = w8 + v8
    and the kernel accumulates THREE DoubleRow products per output tile
      psum = x8@w8 + x8@v8 + r8@w8       (dropped r8@v8 term ~ 7e-4 rel)
    in a single PSUM group — 24 matmuls of 256 cycles per [128,512] tile
    vs bf16's 16x512 = 0.75x the PE time. Eviction fuses the shared
    dequant scale c=1/(sx*sw) and bias: out = psum*c + b in one DVE
    scalar_tensor_tensor. Rel err ~1.5e-3, well inside the 2e-2 gate.

W (both fp8 halves) stays resident in SBUF, DMA'd in per-(tensor, n, ko)
chunks interleaved with the phase-A x tiles so the first matmul waits
only ~1.5 us and the PE tracks delivery; phase B streams the remaining
m-tiles with W fully resident.
"""

import math
import sys

for _p in ("/opt/trn_rl_repo", "/root/.axon_site/_ro/trn_rl_repo"):
    if _p not in sys.path:
        sys.path.append(_p)

import ml_dtypes
import numpy as np

BF16 = np.dtype(ml_dtypes.bfloat16)
F8 = np.dtype(ml_dtypes.float8_e4m3)

from concourse import bacc, mybir, tile
from concourse.bass_utils import run_bass_kernel_spmd

P = 128
D_IN = 2048
D_OUT = 2048
KO = D_IN // P  # 16 k-subtiles
NQ = KO // 2  # 8 DoubleRow k-pair groups
N_TILE = 512
N_TILES = D_OUT // N_TILE  # 4
NUM_GROUPS = 8
N_CORES = 8

MODE = "bal"

_nc_cache: dict = {}


def build_program(C: int, repeat: int = 1, ph_a: int = 4, mode: str | None = None):
    mode = MODE if mode is None else mode
    if mode == "fp8":
        return build_program_fp8(C, repeat=repeat, ph_a=ph_a)
    if mode == "bal":
        m_main, bh, _units = BAL_PLAN
        return build_program_bal(m_main, bh, repeat=repeat, ph_a=ph_a)
    return build_program_bf16(C, repeat=repeat, ph_a=ph_a, tail="narrow")


def build_program_fp8(C: int, repeat: int = 1, ph_a: int = 4):
    """Error-compensated fp8 DoubleRow program for row capacity C."""
    key = ("fp8", C, repeat, ph_a)
    if key in _nc_cache:
        return _nc_cache[key]
    assert C % P == 0
    m_tiles = C // P
    f32 = mybir.dt.float32
    f8 = mybir.dt.float8e4
    DR = mybir.MatmulPerfMode.DoubleRow
    MULT = mybir.AluOpType.mult
    ADD = mybir.AluOpType.add

    nc = bacc.Bacc(
        "TRN2", target_bir_lowering=False, debug=False, num_devices=N_CORES
    )
    # Blocked HBM layouts (prepared host-side) so every DMA moves large
    # contiguous per-partition runs:
    #   xT8/xTr[m, kp, ko, j]  = x8/r8[m*128+j, ko*128+kp]
    #   wT8/wTv[n, kp, ko, nn] = w8/v8^T[ko*128+kp, n*512+nn]
    xT8 = nc.dram_tensor("xT8", [m_tiles, P, KO, P], f8, kind="ExternalInput").ap()
    xTr = nc.dram_tensor("xTr", [m_tiles, P, KO, P], f8, kind="ExternalInput").ap()
    wT8 = nc.dram_tensor("wT8", [N_TILES, P, KO, N_TILE], f8, kind="ExternalInput").ap()
    wTv = nc.dram_tensor("wTv", [N_TILES, P, KO, N_TILE], f8, kind="ExternalInput").ap()
    bb = nc.dram_tensor("bb", [P, D_OUT], f32, kind="ExternalInput").ap()
    cc = nc.dram_tensor("cc", [P, 1], f32, kind="ExternalInput").ap()
    out = nc.dram_tensor("out", [C, D_OUT], f32, kind="ExternalOutput").ap()

    ph_a = min(ph_a, m_tiles)

    with tile.TileContext(nc) as tc:
        with (
            tc.tile_pool(name="wpool", bufs=1) as wpool,
            tc.tile_pool(name="cpool", bufs=1) as cpool,
            tc.tile_pool(name="xapool", bufs=1) as xapool,
            tc.tile_pool(name="xpool", bufs=3) as xpool,
            tc.tile_pool(name="opool", bufs=4) as opool,
            tc.tile_pool(name="ofpool", bufs=2) as ofpool,
            tc.tile_pool(name="pspool", bufs=8, space="PSUM") as pspool,
        ):
            w8_sb = wpool.tile([P, N_TILES, KO, N_TILE], f8, name="w8")
            v8_sb = wpool.tile([P, N_TILES, KO, N_TILE], f8, name="v8")
            b_sb = cpool.tile([P, D_OUT], f32, name="b")
            c_sb = cpool.tile([P, 1], f32, name="c")
            x8a = xapool.tile([P, ph_a, KO, P], f8, name="x8a")
            r8a = xapool.tile([P, ph_a, KO, P], f8, name="r8a")

            KQ = 4  # ko's per W chunk: 2 KB/partition, ~0.7 us transfer
            n_kq = KO // KQ

            def w_chunk(t_sb, t_hbm, n, kq):
                nc.sync.dma_start(
                    t_sb[:, n, kq * KQ : (kq + 1) * KQ],
                    t_hbm[n, :, kq * KQ : (kq + 1) * KQ],
                )

            # DMA issue order ~= HBM service order. The first x8 half-tile
            # and first w8 chunk lead so the first matmul's dependency chain
            # is short; per-(n,kq) w8/v8 chunks then interleave in exactly
            # the order phase A consumes them.
            nc.sync.dma_start(x8a[:, 0, : KO // 2], xT8[0, :, : KO // 2])
            nc.sync.dma_start(w8_sb[:, 0, :KQ], wT8[0, :, :KQ])
            nc.sync.dma_start(x8a[:, 0, KO // 2 :], xT8[0, :, KO // 2 :])
            nc.sync.dma_start(r8a[:, 0], xTr[0])
            w_chunk(v8_sb, wTv, 0, 0)
            if ph_a > 1:
                nc.sync.dma_start(x8a[:, 1], xT8[1])
                nc.sync.dma_start(r8a[:, 1], xTr[1])
            for kq in range(1, n_kq):
                w_chunk(w8_sb, wT8, 0, kq)
                w_chunk(v8_sb, wTv, 0, kq)
            for m in range(2, ph_a):
                nc.sync.dma_start(x8a[:, m], xT8[m])
                nc.sync.dma_start(r8a[:, m], xTr[m])
            nc.sync.dma_start(b_sb[:], bb[:])
            nc.sync.dma_start(c_sb[:], cc[:])
            for n in range(1, N_TILES):
                for kq in range(n_kq):
                    w_chunk(w8_sb, wT8, n, kq)
                    w_chunk(v8_sb, wTv, n, kq)

            def evict(ps, m, n, o_full=None):
                ms = slice(m * P, (m + 1) * P)
                ns = slice(n * N_TILE, (n + 1) * N_TILE)
                # out = psum * c + bias fused on the DVE
                if o_full is None:
                    o_sb = opool.tile([P, N_TILE], f32, tag="o")
                    nc.vector.scalar_tensor_tensor(
                        o_sb[:], ps, c_sb[:], b_sb[:, ns], MULT, ADD
                    )
                    # out DMAs ride the Activation HWDGE queue so their
                    # descriptor generation doesn't serialize behind the
                    # x/W loads on the SP queue
                    nc.scalar.dma_start(out[ms, ns], o_sb[:])
                else:
                    nc.vector.scalar_tensor_tensor(
                        o_full[:, ns], ps, c_sb[:], b_sb[:, ns], MULT, ADD
                    )

            # the three compensated products share one PSUM accumulation
            # group; per k-pair the product order matches W chunk delivery
            # (w8 then v8)
            def products(x8_t, r8_t):
                return ((x8_t, w8_sb), (x8_t, v8_sb), (r8_t, w8_sb))

            def mm_group(ps, x8_t, r8_t, n, q, t, start_q):
                xs, ws = products(x8_t, r8_t)[t]
                nc.tensor.matmul(
                    ps,
                    xs[:, 2 * q : 2 * q + 2],
                    ws[:, n, 2 * q : 2 * q + 2],
                    start=(q == start_q and t == 0),
                    stop=(q == NQ - 1 and t == 2),
                    perf_mode=DR,
                )

            def do_tile(x8_t, r8_t, m, n, o_full=None):
                ps = pspool.tile([P, N_TILE], f32, tag="ps")
                for q in range(NQ):
                    for t in range(3):
                        mm_group(ps, x8_t, r8_t, n, q, t, 0)
                evict(ps, m, n, o_full)

            def phase_a_quarter(n, m_set):
                pss = {}
                for m in m_set:
                    pss[m] = pspool.tile(
                        [P, N_TILE], f32, tag="ps", name=f"psA_{n}_{m}"
                    )
                for q in range(NQ):
                    for t in range(3):
                        for m in m_set:
                            mm_group(
                                pss[m], x8a[:, m], r8a[:, m], n, q, t, 0
                            )
                for m in m_set:
                    evict(pss[m], m, n)

            for rep in range(repeat):
                if rep == 0:
                    # phase A: k-major across resident x tiles per quarter.
                    # n=0 runs in two waves because only two x tiles have
                    # arrived when its chunks start landing.
                    first = list(range(min(2, ph_a)))
                    rest = list(range(min(2, ph_a), ph_a))
                    phase_a_quarter(0, first)
                    if rest:
                        phase_a_quarter(0, rest)
                    for n in range(1, N_TILES):
                        phase_a_quarter(n, list(range(ph_a)))
                    b_start = ph_a
                else:
                    b_start = 0
                # phase B: steady-state streaming; full-row out tiles so the
                # out DMA writes 8 KB/partition contiguous
                for m in range(b_start, m_tiles):
                    x8_sb = xpool.tile([P, KO, P], f8, tag="x8")
                    r8_sb = xpool.tile([P, KO, P], f8, tag="r8")
                    nc.sync.dma_start(x8_sb[:], xT8[m])
                    nc.sync.dma_start(r8_sb[:], xTr[m])
                    if m == m_tiles - 1:
                        # last tile: per-slice eviction so the final out DMA
                        # doesn't serialize behind all 4 evictions
                        for n in range(N_TILES):
                            do_tile(x8_sb, r8_sb, m, n)
                    else:
                        o_full = ofpool.tile([P, D_OUT], f32, tag="of")
                        for n in range(N_TILES):
                            do_tile(x8_sb, r8_sb, m, n, o_full=o_full)
                        nc.scalar.dma_start(
                            out[m * P : (m + 1) * P, :], o_full[:]
                        )

    nc.compile()
    _nc_cache[key] = nc
    return nc


def build_program_bf16(
    C: int,
    repeat: int = 1,
    ph_a: int = 4,
    inner: str = "n",
    out_dt: str = "f32",
    lead: str = "orig",
    tail: str = "orig",
):
    """bf16 program for row capacity C.

    inner: "n"  — per (m, n) psum group, n inner (16-matmul groups)
           "ko" — per m-tile, 4 psum groups interleaved ko-outer (stationary
                  x[:, ko] shared by 4 consecutive matmuls; group-boundary
                  sync amortized 4x)
           "ko2"— per 2 m-tiles, 8 psum groups interleaved ko-outer
    out_dt: "f32" | "bf16" — HBM dtype of out (bf16 halves store traffic;
           host casts back, ~2e-3 extra rounding)
    lead: "fine" — 2-ko first x piece + 1-ko W chunks on the act queue so
          the first matmul starts ~1.8 us earlier; first phase-A wave is
          m={0} alone so it only waits on x tile 0
    tail: "narrow" — last m-tile ends with 256/128/128-wide psum groups so
          the final evict+DMA after the last matmul is ~3x shorter
    """
    key = ("bf16", C, repeat, ph_a, inner, out_dt, lead, tail)
    if key in _nc_cache:
        return _nc_cache[key]
    assert C % P == 0
    m_tiles = C // P
    f32 = mybir.dt.float32
    bf16 = mybir.dt.bfloat16
    odt = f32 if out_dt == "f32" else bf16

    nc = bacc.Bacc(
        "TRN2", target_bir_lowering=False, debug=False, num_devices=N_CORES
    )
    xT = nc.dram_tensor("xT", [m_tiles, P, KO, P], bf16, kind="ExternalInput").ap()
    wT = nc.dram_tensor("wT", [N_TILES, P, KO, N_TILE], bf16, kind="ExternalInput").ap()
    bb = nc.dram_tensor("bb", [P, D_OUT], f32, kind="ExternalInput").ap()
    out = nc.dram_tensor("out", [C, D_OUT], odt, kind="ExternalOutput").ap()

    ph_a = min(ph_a, m_tiles)

    with tile.TileContext(nc) as tc:
        with (
            tc.tile_pool(name="wpool", bufs=1) as wpool,
            tc.tile_pool(name="cpool", bufs=1) as cpool,
            tc.tile_pool(name="xapool", bufs=1) as xapool,
            tc.tile_pool(name="xpool", bufs=3) as xpool,
            tc.tile_pool(name="opool", bufs=4) as opool,
            tc.tile_pool(name="ofpool", bufs=4) as ofpool,
            tc.tile_pool(name="pspool", bufs=8, space="PSUM") as pspool,
        ):
            w_sb = wpool.tile([P, N_TILES, KO, N_TILE], bf16)
            b_sb = cpool.tile([P, D_OUT], f32)
            xa_sb = xapool.tile([P, ph_a, KO, P], bf16)

            KQ = 4
            n_kq = KO // KQ

            def w_chunk(n, kq):
                nc.sync.dma_start(
                    w_sb[:, n, kq * KQ : (kq + 1) * KQ],
                    wT[n, :, kq * KQ : (kq + 1) * KQ],
                )

            def xa_dma(m):
                nc.sync.dma_start(xa_sb[:, m], xT[m])

            if lead == "par":
                # n=0 W rides the act queue in 2/2/4/8-ko pieces, in parallel
                # with x on the sync queue: the first matmul waits only
                # max(x0h1, w0[0:2]) ~= 2 us, and the m={0}-only first wave
                # consumes slowly enough (p-state ramp) to track delivery.
                nc.sync.dma_start(xa_sb[:, 0, : KO // 2], xT[0, :, : KO // 2])
                nc.scalar.dma_start(w_sb[:, 0, :2], wT[0, :, :2])
                nc.scalar.dma_start(w_sb[:, 0, 2:4], wT[0, :, 2:4])
                nc.scalar.dma_start(w_sb[:, 0, 4:8], wT[0, :, 4:8])
                nc.scalar.dma_start(w_sb[:, 0, 8:], wT[0, :, 8:])
                nc.sync.dma_start(xa_sb[:, 0, KO // 2 :], xT[0, :, KO // 2 :])
                for m in range(1, ph_a):
                    xa_dma(m)
                nc.sync.dma_start(b_sb[:], bb[:])
                for n in range(1, N_TILES):
                    for kq in range(n_kq):
                        w_chunk(n, kq)
            elif lead == "fine":
                # x tile 0 in 2/6/8-ko pieces on the sync queue; n=0 W in
                # 1/1/2/4/8-ko pieces on the act queue (parallel DGE) paced
                # just ahead of the ko-loop's consumption. First matmul only
                # waits ~1.8 us for x0[0:2] + w0[0:1].
                nc.sync.dma_start(xa_sb[:, 0, :2], xT[0, :, :2])
                nc.scalar.dma_start(w_sb[:, 0, :1], wT[0, :, :1])
                nc.sync.dma_start(xa_sb[:, 0, 2:8], xT[0, :, 2:8])
                nc.scalar.dma_start(w_sb[:, 0, 1:2], wT[0, :, 1:2])
                nc.sync.dma_start(xa_sb[:, 0, 8:], xT[0, :, 8:])
                nc.scalar.dma_start(w_sb[:, 0, 2:4], wT[0, :, 2:4])
                nc.scalar.dma_start(w_sb[:, 0, 4:8], wT[0, :, 4:8])
                nc.scalar.dma_start(w_sb[:, 0, 8:], wT[0, :, 8:])
                for m in range(1, ph_a):
                    xa_dma(m)
                nc.sync.dma_start(b_sb[:], bb[:])
                for n in range(1, N_TILES):
                    for kq in range(n_kq):
                        w_chunk(n, kq)
            else:
                nc.sync.dma_start(xa_sb[:, 0, : KO // 2], xT[0, :, : KO // 2])
                nc.sync.dma_start(w_sb[:, 0, :2], wT[0, :, :2])
                nc.sync.dma_start(xa_sb[:, 0, KO // 2 :], xT[0, :, KO // 2 :])
                nc.sync.dma_start(w_sb[:, 0, 2:4], wT[0, :, 2:4])
                if ph_a > 1:
                    xa_dma(1)
                for kq in range(1, n_kq):
                    w_chunk(0, kq)
                for m in range(2, ph_a):
                    xa_dma(m)
                nc.sync.dma_start(b_sb[:], bb[:])
                for n in range(1, N_TILES):
                    for kq in range(n_kq):
                        w_chunk(n, kq)

            def evict(ps, m, n):
                ms = slice(m * P, (m + 1) * P)
                ns = slice(n * N_TILE, (n + 1) * N_TILE)
                o_sb = opool.tile([P, N_TILE], odt, tag="o", name=f"o_{m}_{n}")
                nc.vector.tensor_add(o_sb[:], ps, b_sb[:, ns])
                nc.scalar.dma_start(out[ms, ns], o_sb[:])

            def do_group(x_tile, m, n, o_full=None):
                ps = pspool.tile([P, N_TILE], f32, tag="ps")
                for ko in range(KO):
                    nc.tensor.matmul(
                        ps,
                        x_tile[:, ko],
                        w_sb[:, n, ko],
                        start=(ko == 0),
                        stop=(ko == KO - 1),
                    )
                if o_full is None:
                    evict(ps, m, n)
                else:
                    ns = slice(n * N_TILE, (n + 1) * N_TILE)
                    nc.vector.tensor_add(o_full[:, ns], ps, b_sb[:, ns])

            def do_group_cols(x_tile, m, c0, c1):
                """Narrow trailing psum group over out cols [c0, c1) (within
                one n-tile); psum/o_sb are slices of full-width pool tiles."""
                w = c1 - c0
                n = c0 // N_TILE
                ps_full = pspool.tile(
                    [P, N_TILE], f32, tag="ps", name=f"psn_{m}_{c0}"
                )
                ps = ps_full[:, :w]
                for ko in range(KO):
                    nc.tensor.matmul(
                        ps,
                        x_tile[:, ko],
                        w_sb[:, n, ko, c0 - n * N_TILE : c1 - n * N_TILE],
                        start=(ko == 0),
                        stop=(ko == KO - 1),
                    )
                o_sb = opool.tile(
                    [P, N_TILE], odt, tag="o", name=f"on_{m}_{c0}"
                )
                nc.vector.tensor_add(o_sb[:, :w], ps, b_sb[:, c0:c1])
                nc.scalar.dma_start(
                    out[m * P : (m + 1) * P, c0:c1], o_sb[:, :w]
                )

            def do_last_tile(x_tile, m):
                if tail == "narrow":
                    for n in range(N_TILES - 1):
                        do_group(x_tile, m, n)
                    base = (N_TILES - 1) * N_TILE
                    for c0, c1 in (
                        (base, base + 256),
                        (base + 256, base + 384),
                        (base + 384, base + 512),
                    ):
                        do_group_cols(x_tile, m, c0, c1)
                else:
                    for n in range(N_TILES):
                        do_group(x_tile, m, n)

            def do_mtiles_ko_outer(x_tiles, ms, o_fulls):
                """Interleave len(ms)*N_TILES psum groups ko-outer; 4
                consecutive matmuls share the stationary x[:, ko]."""
                pss = {}
                for m in ms:
                    for n in range(N_TILES):
                        pss[m, n] = pspool.tile(
                            [P, N_TILE], f32, tag="ps", name=f"ps_{m}_{n}"
                        )
                for ko in range(KO):
                    for mi, m in enumerate(ms):
                        for n in range(N_TILES):
                            nc.tensor.matmul(
                                pss[m, n],
                                x_tiles[mi][:, ko],
                                w_sb[:, n, ko],
                                start=(ko == 0),
                                stop=(ko == KO - 1),
                            )
                for mi, m in enumerate(ms):
                    for n in range(N_TILES):
                        ns = slice(n * N_TILE, (n + 1) * N_TILE)
                        if o_fulls[mi] is None:
                            evict(pss[m, n], m, n)
                        else:
                            nc.vector.tensor_add(
                                o_fulls[mi][:, ns], pss[m, n], b_sb[:, ns]
                            )

            def phase_a_quarter(n, m_set):
                pss = {}
                for m in m_set:
                    pss[m] = pspool.tile(
                        [P, N_TILE], f32, tag="ps", name=f"psA_{n}_{m}"
                    )
                for ko in range(KO):
                    for m in m_set:
                        nc.tensor.matmul(
                            pss[m],
                            xa_sb[:, m, ko],
                            w_sb[:, n, ko],
                            start=(ko == 0),
                            stop=(ko == KO - 1),
                        )
                for m in m_set:
                    evict(pss[m], m, n)

            for rep in range(repeat):
                if rep == 0:
                    n_first = 1 if lead in ("fine", "par") else min(2, ph_a)
                    first = list(range(n_first))
                    rest = list(range(n_first, ph_a))
                    phase_a_quarter(0, first)
                    if rest:
                        phase_a_quarter(0, rest)
                    for n in range(1, N_TILES):
                        phase_a_quarter(n, list(range(ph_a)))
                    b_start = ph_a
                else:
                    b_start = 0
                if inner == "n":
                    for m in range(b_start, m_tiles):
                        x_sb = xpool.tile([P, KO, P], bf16, tag="x")
                        nc.sync.dma_start(x_sb[:], xT[m])
                        if m == m_tiles - 1:
                            do_last_tile(x_sb, m)
                        else:
                            o_full = ofpool.tile([P, D_OUT], odt, tag="of")
                            for n in range(N_TILES):
                                do_group(x_sb, m, n, o_full=o_full)
                            nc.scalar.dma_start(
                                out[m * P : (m + 1) * P, :], o_full[:]
                            )
                else:
                    step = 2 if inner == "ko2" else 1
                    m = b_start
                    while m < m_tiles:
                        ms = list(range(m, min(m + step, m_tiles)))
                        x_tiles, o_fulls = [], []
                        for mi in ms:
                            x_sb = xpool.tile(
                                [P, KO, P], bf16, tag="x", name=f"x_{mi}"
                            )
                            nc.sync.dma_start(x_sb[:], xT[mi])
                            x_tiles.append(x_sb)
                            if mi == m_tiles - 1:
                                o_fulls.append(None)
                            else:
                                of_sb = ofpool.tile(
                                    [P, D_OUT], odt, tag="of", name=f"of_{mi}"
                                )
                                o_fulls.append(of_sb)
                        do_mtiles_ko_outer(x_tiles, ms, o_fulls)
                        for mi, of in zip(ms, o_fulls):
                            if of is not None:
                                nc.scalar.dma_start(
                                    out[mi * P : (mi + 1) * P, :], of[:]
                                )
                        m += step

    nc.compile()
    _nc_cache[key] = nc
    return nc


NB = 256  # borrow half-unit width (cols)


def build_program_bal(
    m_main: int, bh: int, repeat: int = 1, ph_a: int = 4, lead2: bool = False
):
    """Balanced program: m_main own-expert m-tiles (full D_OUT) plus bh
    borrowed half-units (one x-tile x NB cols each, self-contained W/bias
    slices) redistributing overflow tiles of heavy experts. Perfectly
    balances total padded tile work across the 8 cores.
    """
    key = ("bal", m_main, bh, repeat, ph_a, lead2)
    if key in _nc_cache:
        return _nc_cache[key]
    f32 = mybir.dt.float32
    bf16 = mybir.dt.bfloat16
    C = m_main * P

    nc = bacc.Bacc(
        "TRN2", target_bir_lowering=False, debug=False, num_devices=N_CORES
    )
    xT = nc.dram_tensor("xT", [m_main, P, KO, P], bf16, kind="ExternalInput").ap()
    wT = nc.dram_tensor("wT", [N_TILES, P, KO, N_TILE], bf16, kind="ExternalInput").ap()
    bb = nc.dram_tensor("bb", [P, D_OUT], f32, kind="ExternalInput").ap()
    xB = nc.dram_tensor("xB", [max(bh, 1), P, KO, P], bf16, kind="ExternalInput").ap()
    wB = nc.dram_tensor("wB", [max(bh, 1), P, KO, NB], bf16, kind="ExternalInput").ap()
    bB = nc.dram_tensor("bB", [P, max(bh, 1), NB], f32, kind="ExternalInput").ap()
    out = nc.dram_tensor("out", [C, D_OUT], f32, kind="ExternalOutput").ap()
    outB = nc.dram_tensor(
        "outB", [max(bh, 1) * P, NB], f32, kind="ExternalOutput"
    ).ap()

    ph_a = min(ph_a, m_main)

    with tile.TileContext(nc) as tc:
        with (
            tc.tile_pool(name="wpool", bufs=1) as wpool,
            tc.tile_pool(name="cpool", bufs=1) as cpool,
            tc.tile_pool(name="xapool", bufs=1) as xapool,
            tc.tile_pool(name="xpool", bufs=3) as xpool,
            tc.tile_pool(name="xbpool", bufs=4) as xbpool,
            tc.tile_pool(name="opool", bufs=4) as opool,
            tc.tile_pool(name="ofpool", bufs=3) as ofpool,
            tc.tile_pool(name="pspool", bufs=8, space="PSUM") as pspool,
        ):
            w_sb = wpool.tile([P, N_TILES, KO, N_TILE], bf16, name="w")
            wb_sb = wpool.tile([P, max(bh, 1), KO, NB], bf16, name="wb")
            b_sb = cpool.tile([P, D_OUT], f32, name="b")
            bb_sb = cpool.tile([P, max(bh, 1), NB], f32, name="bb2")
            xa_sb = xapool.tile([P, ph_a, KO, P], bf16, name="xa")

            KQ = 4
            n_kq = KO // KQ

            def w_chunk(n, kq):
                nc.sync.dma_start(
                    w_sb[:, n, kq * KQ : (kq + 1) * KQ],
                    wT[n, :, kq * KQ : (kq + 1) * KQ],
                )

            def xa_dma(m):
                nc.sync.dma_start(xa_sb[:, m], xT[m])

            wq = nc.scalar if lead2 else nc.sync
            nc.sync.dma_start(xa_sb[:, 0, : KO // 2], xT[0, :, : KO // 2])
            wq.dma_start(w_sb[:, 0, :2], wT[0, :, :2])
            nc.sync.dma_start(xa_sb[:, 0, KO // 2 :], xT[0, :, KO // 2 :])
            wq.dma_start(w_sb[:, 0, 2:4], wT[0, :, 2:4])
            if ph_a > 1:
                xa_dma(1)
            for kq in range(1, n_kq):
                (wq if lead2 else nc.sync).dma_start(
                    w_sb[:, 0, kq * KQ : (kq + 1) * KQ],
                    wT[0, :, kq * KQ : (kq + 1) * KQ],
                )
            for m in range(2, ph_a):
                xa_dma(m)
            nc.sync.dma_start(b_sb[:], bb[:])
            for n in range(1, N_TILES):
                for kq in range(n_kq):
                    w_chunk(n, kq)
            for u in range(bh):
                nc.sync.dma_start(wb_sb[:, u], wB[u])
            nc.sync.dma_start(bb_sb[:], bB[:])

            def evict(ps, m, n):
                ms = slice(m * P, (m + 1) * P)
                ns = slice(n * N_TILE, (n + 1) * N_TILE)
                o_sb = opool.tile([P, N_TILE], f32, tag="o", name=f"o_{m}_{n}")
                nc.vector.tensor_add(o_sb[:], ps, b_sb[:, ns])
                nc.scalar.dma_start(out[ms, ns], o_sb[:])

            def do_group(x_tile, m, n, o_full=None):
                ps = pspool.tile([P, N_TILE], f32, tag="ps")
                for ko in range(KO):
                    nc.tensor.matmul(
                        ps,
                        x_tile[:, ko],
                        w_sb[:, n, ko],
                        start=(ko == 0),
                        stop=(ko == KO - 1),
                    )
                if o_full is None:
                    evict(ps, m, n)
                else:
                    ns = slice(n * N_TILE, (n + 1) * N_TILE)
                    nc.vector.tensor_add(o_full[:, ns], ps, b_sb[:, ns])

            def do_borrow(u):
                x_sb = xbpool.tile([P, KO, P], bf16, tag="xb", name=f"xb_{u}")
                nc.sync.dma_start(x_sb[:], xB[u])
                ps_full = pspool.tile([P, N_TILE], f32, tag="ps", name=f"psb_{u}")
                ps = ps_full[:, :NB]
                for ko in range(KO):
                    nc.tensor.matmul(
                        ps,
                        x_sb[:, ko],
                        wb_sb[:, u, ko],
                        start=(ko == 0),
                        stop=(ko == KO - 1),
                    )
                o_sb = opool.tile([P, N_TILE], f32, tag="o", name=f"ob_{u}")
                nc.vector.tensor_add(o_sb[:, :NB], ps, bb_sb[:, u])
                nc.scalar.dma_start(outB[u * P : (u + 1) * P, :], o_sb[:, :NB])

            def phase_a_quarter(n, m_set):
                pss = {}
                for m in m_set:
                    pss[m] = pspool.tile(
                        [P, N_TILE], f32, tag="ps", name=f"psA_{n}_{m}"
                    )
                for ko in range(KO):
                    for m in m_set:
                        nc.tensor.matmul(
                            pss[m],
                            xa_sb[:, m, ko],
                            w_sb[:, n, ko],
                            start=(ko == 0),
                            stop=(ko == KO - 1),
                        )
                for m in m_set:
                    evict(pss[m], m, n)

            for rep in range(repeat):
                if rep == 0:
                    first = list(range(min(2, ph_a)))
                    rest = list(range(min(2, ph_a), ph_a))
                    phase_a_quarter(0, first)
                    if rest:
                        phase_a_quarter(0, rest)
                    for n in range(1, N_TILES):
                        phase_a_quarter(n, list(range(ph_a)))
                    b_start = ph_a
                else:
                    b_start = 0
                for m in range(b_start, m_main):
                    x_sb = xpool.tile([P, KO, P], bf16, tag="x")
                    nc.sync.dma_start(x_sb[:], xT[m])
                    o_full = ofpool.tile([P, D_OUT], f32, tag="of")
                    for n in range(N_TILES):
                        do_group(x_sb, m, n, o_full=o_full)
                    nc.scalar.dma_start(out[m * P : (m + 1) * P, :], o_full[:])
                # borrowed half-units last: short 256-col groups keep the
                # post-last-matmul evict+DMA tail small
                for u in range(bh):
                    do_borrow(u)

    nc.compile()
    _nc_cache[key] = nc
    return nc


def plan_bal(counts):
    """Balanced assignment. Returns (m_main, bh, units) where units[c] is
    the per-core list of (g, mt, n2) borrowed half-units (None = dummy)."""
    tiles = [int(math.ceil(c / 128)) if c else 0 for c in counts]
    m_main = max(1, min(t for t in tiles))
    pool = []
    for g in range(NUM_GROUPS):
        for mt in range(m_main, tiles[g]):
            for n2 in range(D_OUT // NB):
                pool.append((g, mt, n2))
    bh = (len(pool) + N_CORES - 1) // N_CORES
    units = []
    for c in range(N_CORES):
        us = pool[c * bh : (c + 1) * bh]
        us += [None] * (bh - len(us))
        units.append(us)
    return m_main, bh, units


BAL_PLAN = None


def shard_inputs(x, weight, bias, group_indices, mode: str | None = None):
    """Host-side expert-parallel sharding. Returns (in_maps, perm, offsets,
    counts, C)."""
    mode = MODE if mode is None else mode
    n_rows = x.shape[0]
    gi = np.asarray(group_indices)
    perm = np.argsort(gi, kind="stable")
    counts = np.bincount(gi, minlength=NUM_GROUPS).astype(np.int64)
    offsets = np.zeros(NUM_GROUPS + 1, dtype=np.int64)
    np.cumsum(counts, out=offsets[1:])
    C = max(P, int(math.ceil(counts.max() / P)) * P)

    x_sorted = x[perm] if not np.array_equal(perm, np.arange(n_rows)) else x
    m_tiles = C // P

    def block_x(xg):
        # [C, D_IN] -> [m, kp, ko, j] with xT[m, kp, ko, j] = xg[m*128+j, ko*128+kp]
        return np.ascontiguousarray(
            xg.reshape(m_tiles, P, KO, P).transpose(0, 3, 2, 1)
        )

    def block_w(wg):
        # W^T [D_IN, D_OUT] -> [n, kp, ko, nn]
        return np.ascontiguousarray(
            wg.T.reshape(KO, P, N_TILES, N_TILE).transpose(2, 1, 0, 3)
        )

    in_maps = []
    if mode == "bal":
        global BAL_PLAN
        m_main, bh, units = plan_bal(counts)
        BAL_PLAN = (m_main, bh, units)
        Cm = m_main * P

        def block_xg(xg, mt):
            return np.ascontiguousarray(
                xg.reshape(mt, P, KO, P).transpose(0, 3, 2, 1)
            ).astype(BF16)

        for c in range(N_CORES):
            g = c
            ng = int(min(counts[g], Cm))
            xg = np.zeros((Cm, D_IN), dtype=np.float32)
            xg[:ng] = x_sorted[offsets[g] : offsets[g] + ng]
            xBa = np.zeros((max(bh, 1), P, KO, P), dtype=BF16)
            wBa = np.zeros((max(bh, 1), P, KO, NB), dtype=BF16)
            bBa = np.zeros((P, max(bh, 1), NB), dtype=np.float32)
            for u, unit in enumerate(units[c]):
                if unit is None:
                    continue
                gu, mt, n2 = unit
                r0 = mt * P
                nr = int(min(P, counts[gu] - r0))
                xu = np.zeros((P, D_IN), dtype=np.float32)
                xu[:nr] = x_sorted[offsets[gu] + r0 : offsets[gu] + r0 + nr]
                xBa[u] = block_xg(xu, 1)[0]
                wslice = weight[gu].T[:, n2 * NB : (n2 + 1) * NB]
                wBa[u] = np.ascontiguousarray(
                    wslice.reshape(KO, P, NB).transpose(1, 0, 2)
                ).astype(BF16)
                bBa[:, u, :] = bias[gu][n2 * NB : (n2 + 1) * NB]
            in_maps.append(
                {
                    "xT": block_xg(xg, m_main),
                    "wT": block_w(weight[g]).astype(BF16),
                    "bb": np.ascontiguousarray(
                        np.broadcast_to(bias[g], (P, D_OUT))
                    ),
                    "xB": xBa,
                    "wB": wBa,
                    "bB": bBa,
                }
            )
        return in_maps, perm, offsets, counts, C
    if mode == "fp8":
        sx = np.float32(120.0) / max(np.abs(x).max(), 1e-30)
        sw = np.float32(120.0) / max(np.abs(weight).max(), 1e-30)
        c = np.float32(1.0) / (np.float32(sx) * np.float32(sw))
        cc = np.full((P, 1), c, dtype=np.float32)
        for g in range(NUM_GROUPS):
            ng = int(counts[g])
            xg = np.zeros((C, D_IN), dtype=np.float32)
            xg[:ng] = x_sorted[offsets[g] : offsets[g] + ng]
            xs = xg * sx
            x8 = xs.astype(F8)
            r8 = (xs - x8.astype(np.float32)).astype(F8)
            ws = weight[g] * sw
            w8 = ws.astype(F8)
            v8 = (ws - w8.astype(np.float32)).astype(F8)
            in_maps.append(
                {
                    "xT8": block_x(x8),
                    "xTr": block_x(r8),
                    "wT8": block_w(w8),
                    "wTv": block_w(v8),
                    "bb": np.ascontiguousarray(
                        np.broadcast_to(bias[g], (P, D_OUT))
                    ),
                    "cc": cc,
                }
            )
    else:
        for g in range(NUM_GROUPS):
            ng = int(counts[g])
            xg = np.zeros((C, D_IN), dtype=np.float32)
            xg[:ng] = x_sorted[offsets[g] : offsets[g] + ng]
            in_maps.append(
                {
                    "xT": block_x(xg).astype(BF16),
                    "wT": block_w(weight[g]).astype(BF16),
                    "bb": np.ascontiguousarray(
                        np.broadcast_to(bias[g], (P, D_OUT))
                    ),
                }
            )
    return in_maps, perm, offsets, counts, C


def unshard_output(results, perm, offsets, counts, n_rows):
    out = np.empty((n_rows, D_OUT), dtype=np.float32)
    for g in range(NUM_GROUPS):
        ng = int(counts[g])
        out[perm[offsets[g] : offsets[g] + ng]] = results[g]["out"][:ng]
    return out


def unshard_output_bal(results, perm, offsets, counts, n_rows):
    m_main, bh, units = BAL_PLAN
    Cm = m_main * P
    out = np.empty((n_rows, D_OUT), dtype=np.float32)
    for g in range(NUM_GROUPS):
        ng = int(min(counts[g], Cm))
        out[perm[offsets[g] : offsets[g] + ng]] = results[g]["out"][:ng]
    for c in range(N_CORES):
        for u, unit in enumerate(units[c]):
            if unit is None:
                continue
            gu, mt, n2 = unit
            r0 = mt * P
            nr = int(min(P, counts[gu] - r0))
            rows = perm[offsets[gu] + r0 : offsets[gu] + r0 + nr]
            out[rows, n2 * NB : (n2 + 1) * NB] = results[c]["outB"][
                u * P : u * P + nr
            ]
    return out


def kernel(x, weight, bias, group_indices):
    x = np.asarray(x, dtype=np.float32)
    weight = np.asarray(weight, dtype=np.float32)
    bias = np.asarray(bias, dtype=np.float32)
    group_indices = np.asarray(group_indices)
    assert x.shape[1] == D_IN and weight.shape == (NUM_GROUPS, D_OUT, D_IN)

    in_maps, perm, offsets, counts, C = shard_inputs(
        x, weight, bias, group_indices
    )
    nc = build_program(C)
    res = run_bass_kernel_spmd(nc, in_maps, core_ids=list(range(N_CORES)))
    if MODE == "bal":
        return unshard_output_bal(res.results, perm, offsets, counts, x.shape[0])
    return unshard_output(res.results, perm, offsets, counts, x.shape[0])


def _sim_main():
    import tsim

    C = 2176
    for mode in ("fp8", "bf16"):
        _nc_cache.clear()
        nc = build_program(C, repeat=1, mode=mode)
        t1, _ = tsim.simulate(nc)
        nc3 = build_program(C, repeat=3, mode=mode)
        t3, _ = tsim.simulate(nc3)
        body = (t3 - t1) / 2
        print(
            f"{mode}: full {t1 / 1e3:.1f} us, body {body / 1e3:.1f} us,"
            f" overhead {(t1 - body) / 1e3:.1f} us"
        )


if __name__ == "__main__":
    _sim_main()


# revision 27
# speedup vs baseline: 1.0886x; 1.0886x over previous
"""Grouped linear (MoE routing) kernel for 8 Trainium2 NeuronCores.

out[n] = x[n] @ weight[g[n]].T + bias[g[n]]

Strategy: expert-parallel with overflow balancing (MODE="bal").
group_indices is (assumed) sorted; host code computes per-group row
ranges. Core g computes the dense bf16 GEMM for expert g's first
m_main=min_g(tiles_g) 128-row tiles (x and W cast to bf16, fp32 PSUM +
fp32 bias/output, ~2.4e-3 rel err), entirely on-core with no
collectives. The overflow tiles of heavier experts (tiles beyond
m_main) are split into 256-col half-units and redistributed evenly:
each core additionally computes `bh` self-contained borrowed units
(own x-tile + own W[expert][:, n-slice] + bias slice), equalizing the
padded-tile work exactly — per-core PE floor drops from
ceil(max_g)/128 tiles (232.1 us at C=2176) to total/8 (227.0 us).
Host gathers/scatters rows, including borrowed-unit outputs.

Schedule per core: W stays resident in SBUF, DMA'd in per-(n, 4-ko)
chunks interleaved with the phase-A x tiles so the first matmul waits
only ~3.5 us and the PE tracks delivery; phase B streams the remaining
m-tiles (full-row fp32 out tiles, 8 KB/partition DMAs); the borrowed
256-col units run last from a dedicated prefetch pool, which also keeps
the post-last-matmul evict+DMA drain short.

Measured on HW (repeat-differencing, see test.py): steady body ~231 us
vs 227.0 floor; reported full-kernel ~241 us vs 259 us for the
previous C=2176 unbalanced baseline.

Dead ends measured on this HW (kept as modes for reference):
  - fp8 DoubleRow (MODE="fp8"): the cost model's 0.5 cycles/row is
    wrong on HW — a DoubleRow instruction costs the same 512 cycles as
    a bf16 one (2x FLOPs via 2 folded k-subtiles, not 4x). The
    error-compensated 3-product split (x8@w8 + x8@v8 + r8@w8, rel err
    1.1e-3) therefore runs 1.5x SLOWER than bf16 (349 us measured vs
    348.2 predicted at 1.0 c/r). Pure/2-term fp8 is fast enough but
    fails the 2e-2 gate (3.7% / 2.6% rel err).
  - inner="ko"/"ko2" interleaved-psum schedules: no gain (236.7) /
    worse (245.1) vs inner="n" (235.1) — group-boundary sync is already
    hidden; 8-bank-deep interleave adds psum-reuse bubbles.
  - bf16 out store: no gain (DMA is not the bottleneck at 36 MB/core).
  - finer/parallel-queue lead-ins: the original x-half + W-chunk lead
    with a {0,1} first wave is already optimal; starting earlier starves
    the ko loop on W delivery instead.
"""

import math
import sys

for _p in ("/opt/trn_rl_repo", "/root/.axon_site/_ro/trn_rl_repo"):
    if _p not in sys.path:
        sys.path.append(_p)

import ml_dtypes
import numpy as np

BF16 = np.dtype(ml_dtypes.bfloat16)
F8 = np.dtype(ml_dtypes.float8_e4m3)

from concourse import bacc, mybir, tile
from concourse.bass_utils import run_bass_kernel_spmd

P = 128
D_IN = 2048
D_OUT = 2048
KO = D_IN // P  # 16 k-subtiles
NQ = KO // 2  # 8 DoubleRow k-pair groups
N_TILE = 512
N_TILES = D_OUT // N_TILE  # 4
NUM_GROUPS = 8
N_CORES = 8

MODE = "bal"

_nc_cache: dict = {}


def build_program(C: int, repeat: int = 1, ph_a: int = 4, mode: str | None = None):
    mode = MODE if mode is None else mode
    if mode == "fp8":
        return build_program_fp8(C, repeat=repeat, ph_a=ph_a)
    if mode == "bal":
        m_main, bh, _units = BAL_PLAN
        return build_program_bal(m_main, bh, repeat=repeat, ph_a=ph_a)
    return build_program_bf16(C, repeat=repeat, ph_a=ph_a, tail="narrow")


def build_program_fp8(C: int, repeat: int = 1, ph_a: int = 4):
    """Error-compensated fp8 DoubleRow program for row capacity C."""
    key = ("fp8", C, repeat, ph_a)
    if key in _nc_cache:
        return _nc_cache[key]
    assert C % P == 0
    m_tiles = C // P
    f32 = mybir.dt.float32
    f8 = mybir.dt.float8e4
    DR = mybir.MatmulPerfMode.DoubleRow
    MULT = mybir.AluOpType.mult
    ADD = mybir.AluOpType.add

    nc = bacc.Bacc(
        "TRN2", target_bir_lowering=False, debug=False, num_devices=N_CORES
    )
    # Blocked HBM layouts (prepared host-side) so every DMA moves large
    # contiguous per-partition runs:
    #   xT8/xTr[m, kp, ko, j]  = x8/r8[m*128+j, ko*128+kp]
    #   wT8/wTv[n, kp, ko, nn] = w8/v8^T[ko*128+kp, n*512+nn]
    xT8 = nc.dram_tensor("xT8", [m_tiles, P, KO, P], f8, kind="ExternalInput").ap()
    xTr = nc.dram_tensor("xTr", [m_tiles, P, KO, P], f8, kind="ExternalInput").ap()
    wT8 = nc.dram_tensor("wT8", [N_TILES, P, KO, N_TILE], f8, kind="ExternalInput").ap()
    wTv = nc.dram_tensor("wTv", [N_TILES, P, KO, N_TILE], f8, kind="ExternalInput").ap()
    bb = nc.dram_tensor("bb", [P, D_OUT], f32, kind="ExternalInput").ap()
    cc = nc.dram_tensor("cc", [P, 1], f32, kind="ExternalInput").ap()
    out = nc.dram_tensor("out", [C, D_OUT], f32, kind="ExternalOutput").ap()

    ph_a = min(ph_a, m_tiles)

    with tile.TileContext(nc) as tc:
        with (
            tc.tile_pool(name="wpool", bufs=1) as wpool,
            tc.tile_pool(name="cpool", bufs=1) as cpool,
            tc.tile_pool(name="xapool", bufs=1) as xapool,
            tc.tile_pool(name="xpool", bufs=3) as xpool,
            tc.tile_pool(name="opool", bufs=4) as opool,
            tc.tile_pool(name="ofpool", bufs=2) as ofpool,
            tc.tile_pool(name="pspool", bufs=8, space="PSUM") as pspool,
        ):
            w8_sb = wpool.tile([P, N_TILES, KO, N_TILE], f8, name="w8")
            v8_sb = wpool.tile([P, N_TILES, KO, N_TILE], f8, name="v8")
            b_sb = cpool.tile([P, D_OUT], f32, name="b")
            c_sb = cpool.tile([P, 1], f32, name="c")
            x8a = xapool.tile([P, ph_a, KO, P], f8, name="x8a")
            r8a = xapool.tile([P, ph_a, KO, P], f8, name="r8a")

            KQ = 4  # ko's per W chunk: 2 KB/partition, ~0.7 us transfer
            n_kq = KO // KQ

            def w_chunk(t_sb, t_hbm, n, kq):
                nc.sync.dma_start(
                    t_sb[:, n, kq * KQ : (kq + 1) * KQ],
                    t_hbm[n, :, kq * KQ : (kq + 1) * KQ],
                )

            # DMA issue order ~= HBM service order. The first x8 half-tile
            # and first w8 chunk lead so the first matmul's dependency chain
            # is short; per-(n,kq) w8/v8 chunks then interleave in exactly
            # the order phase A consumes them.
            nc.sync.dma_start(x8a[:, 0, : KO // 2], xT8[0, :, : KO // 2])
            nc.sync.dma_start(w8_sb[:, 0, :KQ], wT8[0, :, :KQ])
            nc.sync.dma_start(x8a[:, 0, KO // 2 :], xT8[0, :, KO // 2 :])
            nc.sync.dma_start(r8a[:, 0], xTr[0])
            w_chunk(v8_sb, wTv, 0, 0)
            if ph_a > 1:
                nc.sync.dma_start(x8a[:, 1], xT8[1])
                nc.sync.dma_start(r8a[:, 1], xTr[1])
            for kq in range(1, n_kq):
                w_chunk(w8_sb, wT8, 0, kq)
                w_chunk(v8_sb, wTv, 0, kq)
            for m in range(2, ph_a):
                nc.sync.dma_start(x8a[:, m], xT8[m])
                nc.sync.dma_start(r8a[:, m], xTr[m])
            nc.sync.dma_start(b_sb[:], bb[:])
            nc.sync.dma_start(c_sb[:], cc[:])
            for n in range(1, N_TILES):
                for kq in range(n_kq):
                    w_chunk(w8_sb, wT8, n, kq)
                    w_chunk(v8_sb, wTv, n, kq)

            def evict(ps, m, n, o_full=None):
                ms = slice(m * P, (m + 1) * P)
                ns = slice(n * N_TILE, (n + 1) * N_TILE)
                # out = psum * c + bias fused on the DVE
                if o_full is None:
                    o_sb = opool.tile([P, N_TILE], f32, tag="o")
                    nc.vector.scalar_tensor_tensor(
                        o_sb[:], ps, c_sb[:], b_sb[:, ns], MULT, ADD
                    )
                    # out DMAs ride the Activation HWDGE queue so their
                    # descriptor generation doesn't serialize behind the
                    # x/W loads on the SP queue
                    nc.scalar.dma_start(out[ms, ns], o_sb[:])
                else:
                    nc.vector.scalar_tensor_tensor(
                        o_full[:, ns], ps, c_sb[:], b_sb[:, ns], MULT, ADD
                    )

            # the three compensated products share one PSUM accumulation
            # group; per k-pair the product order matches W chunk delivery
            # (w8 then v8)
            def products(x8_t, r8_t):
                return ((x8_t, w8_sb), (x8_t, v8_sb), (r8_t, w8_sb))

            def mm_group(ps, x8_t, r8_t, n, q, t, start_q):
                xs, ws = products(x8_t, r8_t)[t]
                nc.tensor.matmul(
                    ps,
                    xs[:, 2 * q : 2 * q + 2],
                    ws[:, n, 2 * q : 2 * q + 2],
                    start=(q == start_q and t == 0),
                    stop=(q == NQ - 1 and t == 2),
                    perf_mode=DR,
                )

            def do_tile(x8_t, r8_t, m, n, o_full=None):
                ps = pspool.tile([P, N_TILE], f32, tag="ps")
                for q in range(NQ):
                    for t in range(3):
                        mm_group(ps, x8_t, r8_t, n, q, t, 0)
                evict(ps, m, n, o_full)

            def phase_a_quarter(n, m_set):
                pss = {}
                for m in m_set:
                    pss[m] = pspool.tile(
                        [P, N_TILE], f32, tag="ps", name=f"psA_{n}_{m}"
                    )
                for q in range(NQ):
                    for t in range(3):
                        for m in m_set:
                            mm_group(
                                pss[m], x8a[:, m], r8a[:, m], n, q, t, 0
                            )
                for m in m_set:
                    evict(pss[m], m, n)

            for rep in range(repeat):
                if rep == 0:
                    # phase A: k-major across resident x tiles per quarter.
                    # n=0 runs in two waves because only two x tiles have
                    # arrived when its chunks start landing.
                    first = list(range(min(2, ph_a)))
                    rest = list(range(min(2, ph_a), ph_a))
                    phase_a_quarter(0, first)
                    if rest:
                        phase_a_quarter(0, rest)
                    for n in range(1, N_TILES):
                        phase_a_quarter(n, list(range(ph_a)))
                    b_start = ph_a
                else:
                    b_start = 0
                # phase B: steady-state streaming; full-row out tiles so the
                # out DMA writes 8 KB/partition contiguous
                for m in range(b_start, m_tiles):
                    x8_sb = xpool.tile([P, KO, P], f8, tag="x8")
                    r8_sb = xpool.tile([P, KO, P], f8, tag="r8")
                    nc.sync.dma_start(x8_sb[:], xT8[m])
                    nc.sync.dma_start(r8_sb[:], xTr[m])
                    if m == m_tiles - 1:
                        # last tile: per-slice eviction so the final out DMA
                        # doesn't serialize behind all 4 evictions
                        for n in range(N_TILES):
                            do_tile(x8_sb, r8_sb, m, n)
                    else:
                        o_full = ofpool.tile([P, D_OUT], f32, tag="of")
                        for n in range(N_TILES):
                            do_tile(x8_sb, r8_sb, m, n, o_full=o_full)
                        nc.scalar.dma_start(
                            out[m * P : (m + 1) * P, :], o_full[:]
                        )

    nc.compile()
    _nc_cache[key] = nc
    return nc


def build_program_bf16(
    C: int,
    repeat: int = 1,
    ph_a: int = 4,
    inner: str = "n",
    out_dt: str = "f32",
    lead: str = "orig",
    tail: str = "orig",
):
    """bf16 program for row capacity C.

    inner: "n"  — per (m, n) psum group, n inner (16-matmul groups)
           "ko" — per m-tile, 4 psum groups interleaved ko-outer (stationary
                  x[:, ko] shared by 4 consecutive matmuls; group-boundary
                  sync amortized 4x)
           "ko2"— per 2 m-tiles, 8 psum groups interleaved ko-outer
    out_dt: "f32" | "bf16" — HBM dtype of out (bf16 halves store traffic;
           host casts back, ~2e-3 extra rounding)
    lead: "fine" — 2-ko first x piece + 1-ko W chunks on the act queue so
          the first matmul starts ~1.8 us earlier; first phase-A wave is
          m={0} alone so it only waits on x tile 0
    tail: "narrow" — last m-tile ends with 256/128/128-wide psum groups so
          the final evict+DMA after the last matmul is ~3x shorter
    """
    key = ("bf16", C, repeat, ph_a, inner, out_dt, lead, tail)
    if key in _nc_cache:
        return _nc_cache[key]
    assert C % P == 0
    m_tiles = C // P
    f32 = mybir.dt.float32
    bf16 = mybir.dt.bfloat16
    odt = f32 if out_dt == "f32" else bf16

    nc = bacc.Bacc(
        "TRN2", target_bir_lowering=False, debug=False, num_devices=N_CORES
    )
    xT = nc.dram_tensor("xT", [m_tiles, P, KO, P], bf16, kind="ExternalInput").ap()
    wT = nc.dram_tensor("wT", [N_TILES, P, KO, N_TILE], bf16, kind="ExternalInput").ap()
    bb = nc.dram_tensor("bb", [P, D_OUT], f32, kind="ExternalInput").ap()
    out = nc.dram_tensor("out", [C, D_OUT], odt, kind="ExternalOutput").ap()

    ph_a = min(ph_a, m_tiles)

    with tile.TileContext(nc) as tc:
        with (
            tc.tile_pool(name="wpool", bufs=1) as wpool,
            tc.tile_pool(name="cpool", bufs=1) as cpool,
            tc.tile_pool(name="xapool", bufs=1) as xapool,
            tc.tile_pool(name="xpool", bufs=3) as xpool,
            tc.tile_pool(name="opool", bufs=4) as opool,
            tc.tile_pool(name="ofpool", bufs=4) as ofpool,
            tc.tile_pool(name="pspool", bufs=8, space="PSUM") as pspool,
        ):
            w_sb = wpool.tile([P, N_TILES, KO, N_TILE], bf16)
            b_sb = cpool.tile([P, D_OUT], f32)
            xa_sb = xapool.tile([P, ph_a, KO, P], bf16)

            KQ = 4
            n_kq = KO // KQ

            def w_chunk(n, kq):
                nc.sync.dma_start(
                    w_sb[:, n, kq * KQ : (kq + 1) * KQ],
                    wT[n, :, kq * KQ : (kq + 1) * KQ],
                )

            def xa_dma(m):
                nc.sync.dma_start(xa_sb[:, m], xT[m])

            if lead == "par":
                # n=0 W rides the act queue in 2/2/4/8-ko pieces, in parallel
                # with x on the sync queue: the first matmul waits only
                # max(x0h1, w0[0:2]) ~= 2 us, and the m={0}-only first wave
                # consumes slowly enough (p-state ramp) to track delivery.
                nc.sync.dma_start(xa_sb[:, 0, : KO // 2], xT[0, :, : KO // 2])
                nc.scalar.dma_start(w_sb[:, 0, :2], wT[0, :, :2])
                nc.scalar.dma_start(w_sb[:, 0, 2:4], wT[0, :, 2:4])
                nc.scalar.dma_start(w_sb[:, 0, 4:8], wT[0, :, 4:8])
                nc.scalar.dma_start(w_sb[:, 0, 8:], wT[0, :, 8:])
                nc.sync.dma_start(xa_sb[:, 0, KO // 2 :], xT[0, :, KO // 2 :])
                for m in range(1, ph_a):
                    xa_dma(m)
                nc.sync.dma_start(b_sb[:], bb[:])
                for n in range(1, N_TILES):
                    for kq in range(n_kq):
                        w_chunk(n, kq)
            elif lead == "fine":
                # x tile 0 in 2/6/8-ko pieces on the sync queue; n=0 W in
                # 1/1/2/4/8-ko pieces on the act queue (parallel DGE) paced
                # just ahead of the ko-loop's consumption. First matmul only
                # waits ~1.8 us for x0[0:2] + w0[0:1].
                nc.sync.dma_start(xa_sb[:, 0, :2], xT[0, :, :2])
                nc.scalar.dma_start(w_sb[:, 0, :1], wT[0, :, :1])
                nc.sync.dma_start(xa_sb[:, 0, 2:8], xT[0, :, 2:8])
                nc.scalar.dma_start(w_sb[:, 0, 1:2], wT[0, :, 1:2])
                nc.sync.dma_start(xa_sb[:, 0, 8:], xT[0, :, 8:])
                nc.scalar.dma_start(w_sb[:, 0, 2:4], wT[0, :, 2:4])
                nc.scalar.dma_start(w_sb[:, 0, 4:8], wT[0, :, 4:8])
                nc.scalar.dma_start(w_sb[:, 0, 8:], wT[0, :, 8:])
                for m in range(1, ph_a):
                    xa_dma(m)
                nc.sync.dma_start(b_sb[:], bb[:])
                for n in range(1, N_TILES):
                    for kq in range(n_kq):
                        w_chunk(n, kq)
            else:
                nc.sync.dma_start(xa_sb[:, 0, : KO // 2], xT[0, :, : KO // 2])
                nc.sync.dma_start(w_sb[:, 0, :2], wT[0, :, :2])
                nc.sync.dma_start(xa_sb[:, 0, KO // 2 :], xT[0, :, KO // 2 :])
                nc.sync.dma_start(w_sb[:, 0, 2:4], wT[0, :, 2:4])
                if ph_a > 1:
                    xa_dma(1)
                for kq in range(1, n_kq):
                    w_chunk(0, kq)
                for m in range(2, ph_a):
                    xa_dma(m)
                nc.sync.dma_start(b_sb[:], bb[:])
                for n in range(1, N_TILES):
                    for kq in range(n_kq):
                        w_chunk(n, kq)

            def evict(ps, m, n):
                ms = slice(m * P, (m + 1) * P)
                ns = slice(n * N_TILE, (n + 1) * N_TILE)
                o_sb = opool.tile([P, N_TILE], odt, tag="o", name=f"o_{m}_{n}")
                nc.vector.tensor_add(o_sb[:], ps, b_sb[:, ns])
                nc.scalar.dma_start(out[ms, ns], o_sb[:])

            def do_group(x_tile, m, n, o_full=None):
                ps = pspool.tile([P, N_TILE], f32, tag="ps")
                for ko in range(KO):
                    nc.tensor.matmul(
                        ps,
                        x_tile[:, ko],
                        w_sb[:, n, ko],
                        start=(ko == 0),
                        stop=(ko == KO - 1),
                    )
                if o_full is None:
                    evict(ps, m, n)
                else:
                    ns = slice(n * N_TILE, (n + 1) * N_TILE)
                    nc.vector.tensor_add(o_full[:, ns], ps, b_sb[:, ns])

            def do_group_cols(x_tile, m, c0, c1):
                """Narrow trailing psum group over out cols [c0, c1) (within
                one n-tile); psum/o_sb are slices of full-width pool tiles."""
                w = c1 - c0
                n = c0 // N_TILE
                ps_full = pspool.tile(
                    [P, N_TILE], f32, tag="ps", name=f"psn_{m}_{c0}"
                )
                ps = ps_full[:, :w]
                for ko in range(KO):
                    nc.tensor.matmul(
                        ps,
                        x_tile[:, ko],
                        w_sb[:, n, ko, c0 - n * N_TILE : c1 - n * N_TILE],
                        start=(ko == 0),
                        stop=(ko == KO - 1),
                    )
                o_sb = opool.tile(
                    [P, N_TILE], odt, tag="o", name=f"on_{m}_{c0}"
                )
                nc.vector.tensor_add(o_sb[:, :w], ps, b_sb[:, c0:c1])
                nc.scalar.dma_start(
                    out[m * P : (m + 1) * P, c0:c1], o_sb[:, :w]
                )

            def do_last_tile(x_tile, m):
                if tail == "narrow":
                    for n in range(N_TILES - 1):
                        do_group(x_tile, m, n)
                    base = (N_TILES - 1) * N_TILE
                    for c0, c1 in (
                        (base, base + 256),
                        (base + 256, base + 384),
                        (base + 384, base + 512),
                    ):
                        do_group_cols(x_tile, m, c0, c1)
                else:
                    for n in range(N_TILES):
                        do_group(x_tile, m, n)

            def do_mtiles_ko_outer(x_tiles, ms, o_fulls):
                """Interleave len(ms)*N_TILES psum groups ko-outer; 4
                consecutive matmuls share the stationary x[:, ko]."""
                pss = {}
                for m in ms:
                    for n in range(N_TILES):
                        pss[m, n] = pspool.tile(
                            [P, N_TILE], f32, tag="ps", name=f"ps_{m}_{n}"
                        )
                for ko in range(KO):
                    for mi, m in enumerate(ms):
                        for n in range(N_TILES):
                            nc.tensor.matmul(
                                pss[m, n],
                                x_tiles[mi][:, ko],
                                w_sb[:, n, ko],
                                start=(ko == 0),
                                stop=(ko == KO - 1),
                            )
                for mi, m in enumerate(ms):
                    for n in range(N_TILES):
                        ns = slice(n * N_TILE, (n + 1) * N_TILE)
                        if o_fulls[mi] is None:
                            evict(pss[m, n], m, n)
                        else:
                            nc.vector.tensor_add(
                                o_fulls[mi][:, ns], pss[m, n], b_sb[:, ns]
                            )

            def phase_a_quarter(n, m_set):
                pss = {}
                for m in m_set:
                    pss[m] = pspool.tile(
                        [P, N_TILE], f32, tag="ps", name=f"psA_{n}_{m}"
                    )
                for ko in range(KO):
                    for m in m_set:
                        nc.tensor.matmul(
                            pss[m],
                            xa_sb[:, m, ko],
                            w_sb[:, n, ko],
                            start=(ko == 0),
                            stop=(ko == KO - 1),
                        )
                for m in m_set:
                    evict(pss[m], m, n)

            for rep in range(repeat):
                if rep == 0:
                    n_first = 1 if lead in ("fine", "par") else min(2, ph_a)
                    first = list(range(n_first))
                    rest = list(range(n_first, ph_a))
                    phase_a_quarter(0, first)
                    if rest:
                        phase_a_quarter(0, rest)
                    for n in range(1, N_TILES):
                        phase_a_quarter(n, list(range(ph_a)))
                    b_start = ph_a
                else:
                    b_start = 0
                if inner == "n":
                    for m in range(b_start, m_tiles):
                        x_sb = xpool.tile([P, KO, P], bf16, tag="x")
                        nc.sync.dma_start(x_sb[:], xT[m])
                        if m == m_tiles - 1:
                            do_last_tile(x_sb, m)
                        else:
                            o_full = ofpool.tile([P, D_OUT], odt, tag="of")
                            for n in range(N_TILES):
                                do_group(x_sb, m, n, o_full=o_full)
                            nc.scalar.dma_start(
                                out[m * P : (m + 1) * P, :], o_full[:]
                            )
                else:
                    step = 2 if inner == "ko2" else 1
                    m = b_start
                    while m < m_tiles:
                        ms = list(range(m, min(m + step, m_tiles)))
                        x_tiles, o_fulls = [], []
                        for mi in ms:
                            x_sb = xpool.tile(
                                [P, KO, P], bf16, tag="x", name=f"x_{mi}"
                            )
                            nc.sync.dma_start(x_sb[:], xT[mi])
                            x_tiles.append(x_sb)
                            if mi == m_tiles - 1:
                                o_fulls.append(None)
                            else:
                                of_sb = ofpool.tile(
                                    [P, D_OUT], odt, tag="of", name=f"of_{mi}"
                                )
                                o_fulls.append(of_sb)
                        do_mtiles_ko_outer(x_tiles, ms, o_fulls)
                        for mi, of in zip(ms, o_fulls):
                            if of is not None:
                                nc.scalar.dma_start(
                                    out[mi * P : (mi + 1) * P, :], of[:]
                                )
                        m += step

    nc.compile()
    _nc_cache[key] = nc
    return nc


NB = 256  # borrow half-unit width (cols)


def build_program_bal(
    m_main: int, bh: int, repeat: int = 1, ph_a: int = 4, lead2: bool = False
):
    """Balanced program: m_main own-expert m-tiles (full D_OUT) plus bh
    borrowed half-units (one x-tile x NB cols each, self-contained W/bias
    slices) redistributing overflow tiles of heavy experts. Perfectly
    balances total padded tile work across the 8 cores.
    """
    key = ("bal", m_main, bh, repeat, ph_a, lead2)
    if key in _nc_cache:
        return _nc_cache[key]
    f32 = mybir.dt.float32
    bf16 = mybir.dt.bfloat16
    C = m_main * P

    nc = bacc.Bacc(
        "TRN2", target_bir_lowering=False, debug=False, num_devices=N_CORES
    )
    xT = nc.dram_tensor("xT", [m_main, P, KO, P], bf16, kind="ExternalInput").ap()
    wT = nc.dram_tensor("wT", [N_TILES, P, KO, N_TILE], bf16, kind="ExternalInput").ap()
    bb = nc.dram_tensor("bb", [P, D_OUT], f32, kind="ExternalInput").ap()
    xB = nc.dram_tensor("xB", [max(bh, 1), P, KO, P], bf16, kind="ExternalInput").ap()
    wB = nc.dram_tensor("wB", [max(bh, 1), P, KO, NB], bf16, kind="ExternalInput").ap()
    bB = nc.dram_tensor("bB", [P, max(bh, 1), NB], f32, kind="ExternalInput").ap()
    out = nc.dram_tensor("out", [C, D_OUT], f32, kind="ExternalOutput").ap()
    outB = nc.dram_tensor(
        "outB", [max(bh, 1) * P, NB], f32, kind="ExternalOutput"
    ).ap()

    ph_a = min(ph_a, m_main)

    with tile.TileContext(nc) as tc:
        with (
            tc.tile_pool(name="wpool", bufs=1) as wpool,
            tc.tile_pool(name="cpool", bufs=1) as cpool,
            tc.tile_pool(name="xapool", bufs=1) as xapool,
            tc.tile_pool(name="xpool", bufs=3) as xpool,
            tc.tile_pool(name="xbpool", bufs=4) as xbpool,
            tc.tile_pool(name="opool", bufs=4) as opool,
            tc.tile_pool(name="ofpool", bufs=3) as ofpool,
            tc.tile_pool(name="pspool", bufs=8, space="PSUM") as pspool,
        ):
            w_sb = wpool.tile([P, N_TILES, KO, N_TILE], bf16, name="w")
            wb_sb = wpool.tile([P, max(bh, 1), KO, NB], bf16, name="wb")
            b_sb = cpool.tile([P, D_OUT], f32, name="b")
            bb_sb = cpool.tile([P, max(bh, 1), NB], f32, name="bb2")
            xa_sb = xapool.tile([P, ph_a, KO, P], bf16, name="xa")

            KQ = 4
            n_kq = KO // KQ

            def w_chunk(n, kq):
                nc.sync.dma_start(
                    w_sb[:, n, kq * KQ : (kq + 1) * KQ],
                    wT[n, :, kq * KQ : (kq + 1) * KQ],
                )

            def xa_dma(m):
                nc.sync.dma_start(xa_sb[:, m], xT[m])

            wq = nc.scalar if lead2 else nc.sync
            nc.sync.dma_start(xa_sb[:, 0, : KO // 2], xT[0, :, : KO // 2])
            wq.dma_start(w_sb[:, 0, :2], wT[0, :, :2])
            nc.sync.dma_start(xa_sb[:, 0, KO // 2 :], xT[0, :, KO // 2 :])
            wq.dma_start(w_sb[:, 0, 2:4], wT[0, :, 2:4])
            if ph_a > 1:
                xa_dma(1)
            for kq in range(1, n_kq):
                (wq if lead2 else nc.sync).dma_start(
                    w_sb[:, 0, kq * KQ : (kq + 1) * KQ],
                    wT[0, :, kq * KQ : (kq + 1) * KQ],
                )
            for m in range(2, ph_a):
                xa_dma(m)
            nc.sync.dma_start(b_sb[:], bb[:])
            for n in range(1, N_TILES):
                for kq in range(n_kq):
                    w_chunk(n, kq)
            for u in range(bh):
                nc.sync.dma_start(wb_sb[:, u], wB[u])
            nc.sync.dma_start(bb_sb[:], bB[:])

            def evict(ps, m, n):
                ms = slice(m * P, (m + 1) * P)
                ns = slice(n * N_TILE, (n + 1) * N_TILE)
                o_sb = opool.tile([P, N_TILE], f32, tag="o", name=f"o_{m}_{n}")
                nc.vector.tensor_add(o_sb[:], ps, b_sb[:, ns])
                nc.scalar.dma_start(out[ms, ns], o_sb[:])

            def do_group(x_tile, m, n, o_full=None):
                ps = pspool.tile([P, N_TILE], f32, tag="ps")
                for ko in range(KO):
                    nc.tensor.matmul(
                        ps,
                        x_tile[:, ko],
                        w_sb[:, n, ko],
                        start=(ko == 0),
                        stop=(ko == KO - 1),
                    )
                if o_full is None:
                    evict(ps, m, n)
                else:
                    ns = slice(n * N_TILE, (n + 1) * N_TILE)
                    nc.vector.tensor_add(o_full[:, ns], ps, b_sb[:, ns])

            def do_borrow(u):
                x_sb = xbpool.tile([P, KO, P], bf16, tag="xb", name=f"xb_{u}")
                nc.sync.dma_start(x_sb[:], xB[u])
                ps_full = pspool.tile([P, N_TILE], f32, tag="ps", name=f"psb_{u}")
                ps = ps_full[:, :NB]
                for ko in range(KO):
                    nc.tensor.matmul(
                        ps,
                        x_sb[:, ko],
                        wb_sb[:, u, ko],
                        start=(ko == 0),
                        stop=(ko == KO - 1),
                    )
                o_sb = opool.tile([P, N_TILE], f32, tag="o", name=f"ob_{u}")
                nc.vector.tensor_add(o_sb[:, :NB], ps, bb_sb[:, u])
                nc.scalar.dma_start(outB[u * P : (u + 1) * P, :], o_sb[:, :NB])

            def phase_a_quarter(n, m_set):
                pss = {}
                for m in m_set:
                    pss[m] = pspool.tile(
                        [P, N_TILE], f32, tag="ps", name=f"psA_{n}_{m}"
                    )
                for ko in range(KO):
                    for m in m_set:
                        nc.tensor.matmul(
                            pss[m],
                            xa_sb[:, m, ko],
                            w_sb[:, n, ko],
                            start=(ko == 0),
                            stop=(ko == KO - 1),
                        )
                for m in m_set:
                    evict(pss[m], m, n)

            for rep in range(repeat):
                if rep == 0:
                    first = list(range(min(2, ph_a)))
                    rest = list(range(min(2, ph_a), ph_a))
                    phase_a_quarter(0, first)
                    if rest:
                        phase_a_quarter(0, rest)
                    for n in range(1, N_TILES):
                        phase_a_quarter(n, list(range(ph_a)))
                    b_start = ph_a
                else:
                    b_start = 0
                for m in range(b_start, m_main):
                    x_sb = xpool.tile([P, KO, P], bf16, tag="x")
                    nc.sync.dma_start(x_sb[:], xT[m])
                    o_full = ofpool.tile([P, D_OUT], f32, tag="of")
                    for n in range(N_TILES):
                        do_group(x_sb, m, n, o_full=o_full)
                    nc.scalar.dma_start(out[m * P : (m + 1) * P, :], o_full[:])
                # borrowed half-units last: short 256-col groups keep the
                # post-last-matmul evict+DMA tail small
                for u in range(bh):
                    do_borrow(u)

    nc.compile()
    _nc_cache[key] = nc
    return nc


def plan_bal(counts):
    """Balanced assignment. Returns (m_main, bh, units) where units[c] is
    the per-core list of (g, mt, n2) borrowed half-units (None = dummy)."""
    tiles = [int(math.ceil(c / 128)) if c else 0 for c in counts]
    m_main = max(1, min(t for t in tiles))
    pool = []
    for g in range(NUM_GROUPS):
        for mt in range(m_main, tiles[g]):
            for n2 in range(D_OUT // NB):
                pool.append((g, mt, n2))
    bh = (len(pool) + N_CORES - 1) // N_CORES
    units = []
    for c in range(N_CORES):
        us = pool[c * bh : (c + 1) * bh]
        us += [None] * (bh - len(us))
        units.append(us)
    return m_main, bh, units


BAL_PLAN = None


def shard_inputs(x, weight, bias, group_indices, mode: str | None = None):
    """Host-side expert-parallel sharding. Returns (in_maps, perm, offsets,
    counts, C)."""
    mode = MODE if mode is None else mode
    n_rows = x.shape[0]
    gi = np.asarray(group_indices)
    perm = np.argsort(gi, kind="stable")
    counts = np.bincount(gi, minlength=NUM_GROUPS).astype(np.int64)
    offsets = np.zeros(NUM_GROUPS + 1, dtype=np.int64)
    np.cumsum(counts, out=offsets[1:])
    C = max(P, int(math.ceil(counts.max() / P)) * P)

    x_sorted = x[perm] if not np.array_equal(perm, np.arange(n_rows)) else x
    m_tiles = C // P

    def block_x(xg):
        # [C, D_IN] -> [m, kp, ko, j] with xT[m, kp, ko, j] = xg[m*128+j, ko*128+kp]
        return np.ascontiguousarray(
            xg.reshape(m_tiles, P, KO, P).transpose(0, 3, 2, 1)
        )

    def block_w(wg):
        # W^T [D_IN, D_OUT] -> [n, kp, ko, nn]
        return np.ascontiguousarray(
            wg.T.reshape(KO, P, N_TILES, N_TILE).transpose(2, 1, 0, 3)
        )

    in_maps = []
    if mode == "bal":
        global BAL_PLAN
        m_main, bh, units = plan_bal(counts)
        BAL_PLAN = (m_main, bh, units)
        Cm = m_main * P

        def block_xg(xg, mt):
            return np.ascontiguousarray(
                xg.reshape(mt, P, KO, P).transpose(0, 3, 2, 1)
            ).astype(BF16)

        for c in range(N_CORES):
            g = c
            ng = int(min(counts[g], Cm))
            xg = np.zeros((Cm, D_IN), dtype=np.float32)
            xg[:ng] = x_sorted[offsets[g] : offsets[g] + ng]
            xBa = np.zeros((max(bh, 1), P, KO, P), dtype=BF16)
            wBa = np.zeros((max(bh, 1), P, KO, NB), dtype=BF16)
            bBa = np.zeros((P, max(bh, 1), NB), dtype=np.float32)
            for u, unit in enumerate(units[c]):
                if unit is None:
                    continue
                gu, mt, n2 = unit
                r0 = mt * P
                nr = int(min(P, counts[gu] - r0))
                xu = np.zeros((P, D_IN), dtype=np.float32)
                xu[:nr] = x_sorted[offsets[gu] + r0 : offsets[gu] + r0 + nr]
                xBa[u] = block_xg(xu, 1)[0]
                wslice = weight[gu].T[:, n2 * NB : (n2 + 1) * NB]
                wBa[u] = np.ascontiguousarray(
                    wslice.reshape(KO, P, NB).transpose(1, 0, 2)
                ).astype(BF16)
                bBa[:, u, :] = bias[gu][n2 * NB : (n2 + 1) * NB]
            in_maps.append(
                {
                    "xT": block_xg(xg, m_main),
                    "wT": block_w(weight[g]).astype(BF16),
                    "bb": np.ascontiguousarray(
                        np.broadcast_to(bias[g], (P, D_OUT))
                    ),
                    "xB": xBa,
                    "wB": wBa,
                    "bB": bBa,
                }
            )
        return in_maps, perm, offsets, counts, C
    if mode == "fp8":
        sx = np.float32(120.0) / max(np.abs(x).max(), 1e-30)
        sw = np.float32(120.0) / max(np.abs(weight).max(), 1e-30)
        c = np.float32(1.0) / (np.float32(sx) * np.float32(sw))
        cc = np.full((P, 1), c, dtype=np.float32)
        for g in range(NUM_GROUPS):
            ng = int(counts[g])
            xg = np.zeros((C, D_IN), dtype=np.float32)
            xg[:ng] = x_sorted[offsets[g] : offsets[g] + ng]
            xs = xg * sx
            x8 = xs.astype(F8)
            r8 = (xs - x8.astype(np.float32)).astype(F8)
            ws = weight[g] * sw
            w8 = ws.astype(F8)
            v8 = (ws - w8.astype(np.float32)).astype(F8)
            in_maps.append(
                {
                    "xT8": block_x(x8),
                    "xTr": block_x(r8),
                    "wT8": block_w(w8),
                    "wTv": block_w(v8),
                    "bb": np.ascontiguousarray(
                        np.broadcast_to(bias[g], (P, D_OUT))
                    ),
                    "cc": cc,
                }
            )
    else:
        for g in range(NUM_GROUPS):
            ng = int(counts[g])
            xg = np.zeros((C, D_IN), dtype=np.float32)
            xg[:ng] = x_sorted[offsets[g] : offsets[g] + ng]
            in_maps.append(
                {
                    "xT": block_x(xg).astype(BF16),
                    "wT": block_w(weight[g]).astype(BF16),
                    "bb": np.ascontiguousarray(
                        np.broadcast_to(bias[g], (P, D_OUT))
                    ),
                }
            )
    return in_maps, perm, offsets, counts, C


def unshard_output(results, perm, offsets, counts, n_rows):
    out = np.empty((n_rows, D_OUT), dtype=np.float32)
    for g in range(NUM_GROUPS):
        ng = int(counts[g])
        out[perm[offsets[g] : offsets[g] + ng]] = results[g]["out"][:ng]
    return out


def unshard_output_bal(results, perm, offsets, counts, n_rows):
    m_main, bh, units = BAL_PLAN
    Cm = m_main * P
    out = np.empty((n_rows, D_OUT), dtype=np.float32)
    for g in range(NUM_GROUPS):
        ng = int(min(counts[g], Cm))
        out[perm[offsets[g] : offsets[g] + ng]] = results[g]["out"][:ng]
    for c in range(N_CORES):
        for u, unit in enumerate(units[c]):
            if unit is None:
                continue
            gu, mt, n2 = unit
            r0 = mt * P
            nr = int(min(P, counts[gu] - r0))
            rows = perm[offsets[gu] + r0 : offsets[gu] + r0 + nr]
            out[rows, n2 * NB : (n2 + 1) * NB] = results[c]["outB"][
                u * P : u * P + nr
            ]
    return out


def kernel(x, weight, bias, group_indices):
    x = np.asarray(x, dtype=np.float32)
    weight = np.asarray(weight, dtype=np.float32)
    bias = np.asarray(bias, dtype=np.float32)
    group_indices = np.asarray(group_indices)
    assert x.shape[1] == D_IN and weight.shape == (NUM_GROUPS, D_OUT, D_IN)

    in_maps, perm, offsets, counts, C = shard_inputs(
        x, weight, bias, group_indices
    )
    nc = build_program(C)
    res = run_bass_kernel_spmd(nc, in_maps, core_ids=list(range(N_CORES)))
    if MODE == "bal":
        return unshard_output_bal(res.results, perm, offsets, counts, x.shape[0])
    return unshard_output(res.results, perm, offsets, counts, x.shape[0])


def _sim_main():
    import tsim

    C = 2176
    for mode in ("fp8", "bf16"):
        _nc_cache.clear()
        nc = build_program(C, repeat=1, mode=mode)
        t1, _ = tsim.simulate(nc)
        nc3 = build_program(C, repeat=3, mode=mode)
        t3, _ = tsim.simulate(nc3)
        body = (t3 - t1) / 2
        print(
            f"{mode}: full {t1 / 1e3:.1f} us, body {body / 1e3:.1f} us,"
            f" overhead {(t1 - body) / 1e3:.1f} us"
        )


if __name__ == "__main__":
    _sim_main()


# revision 29
# speedup vs baseline: 1.0944x; 1.0053x over previous
"""Grouped linear (MoE routing) kernel for 8 Trainium2 NeuronCores.

out[n] = x[n] @ weight[g[n]].T + bias[g[n]]

Strategy: expert-parallel with overflow balancing (MODE="bal").
group_indices is (assumed) sorted; host code computes per-group row
ranges. Core g computes the dense bf16 GEMM for expert g's first
m_main=min_g(tiles_g) 128-row tiles (x and W cast to bf16, fp32 PSUM +
fp32 bias/output, ~2.4e-3 rel err), entirely on-core with no
collectives. The overflow tiles of heavier experts (tiles beyond
m_main) are split into 256-col half-units and redistributed evenly:
each core additionally computes `bh` self-contained borrowed units
(own x-tile + own W[expert][:, n-slice] + bias slice), equalizing the
padded-tile work exactly — per-core PE floor drops from
ceil(max_g)/128 tiles (232.1 us at C=2176) to total/8 (227.0 us).
Host gathers/scatters rows, including borrowed-unit outputs.

Schedule per core: W stays resident in SBUF, DMA'd in per-(n, 4-ko)
chunks interleaved with the phase-A x tiles so the first matmul waits
only ~3.5 us and the PE tracks delivery; phase B streams the remaining
m-tiles (full-row fp32 out tiles, 8 KB/partition DMAs); the borrowed
256-col units run last from a dedicated prefetch pool, which also keeps
the post-last-matmul evict+DMA drain short.

Measured on HW (repeat-differencing, see test.py): steady body ~231 us
vs 227.0 floor; reported full-kernel ~241 us vs 259 us for the
previous C=2176 unbalanced baseline.

Dead ends measured on this HW (kept as modes for reference):
  - fp8 DoubleRow (MODE="fp8"): the cost model's 0.5 cycles/row is
    wrong on HW — a DoubleRow instruction costs the same 512 cycles as
    a bf16 one (2x FLOPs via 2 folded k-subtiles, not 4x). The
    error-compensated 3-product split (x8@w8 + x8@v8 + r8@w8, rel err
    1.1e-3) therefore runs 1.5x SLOWER than bf16 (349 us measured vs
    348.2 predicted at 1.0 c/r). Pure/2-term fp8 is fast enough but
    fails the 2e-2 gate (3.7% / 2.6% rel err).
  - inner="ko"/"ko2" interleaved-psum schedules: no gain (236.7) /
    worse (245.1) vs inner="n" (235.1) — group-boundary sync is already
    hidden; 8-bank-deep interleave adds psum-reuse bubbles.
  - bf16 out store: no gain (DMA is not the bottleneck at 36 MB/core).
  - finer/parallel-queue lead-ins: the original x-half + W-chunk lead
    with a {0,1} first wave is already optimal; starting earlier starves
    the ko loop on W delivery instead.
"""

import math
import sys

for _p in ("/opt/trn_rl_repo", "/root/.axon_site/_ro/trn_rl_repo"):
    if _p not in sys.path:
        sys.path.append(_p)

import ml_dtypes
import numpy as np

BF16 = np.dtype(ml_dtypes.bfloat16)
F8 = np.dtype(ml_dtypes.float8_e4m3)

from concourse import bacc, mybir, tile
from concourse.bass_utils import run_bass_kernel_spmd

P = 128
D_IN = 2048
D_OUT = 2048
KO = D_IN // P  # 16 k-subtiles
NQ = KO // 2  # 8 DoubleRow k-pair groups
N_TILE = 512
N_TILES = D_OUT // N_TILE  # 4
NUM_GROUPS = 8
N_CORES = 8

MODE = "bal"

_nc_cache: dict = {}


def build_program(C: int, repeat: int = 1, ph_a: int = 4, mode: str | None = None):
    mode = MODE if mode is None else mode
    if mode == "fp8":
        return build_program_fp8(C, repeat=repeat, ph_a=ph_a)
    if mode == "bal":
        m_main, bh, _units = BAL_PLAN
        return build_program_bal(m_main, bh, repeat=repeat, ph_a=ph_a)
    return build_program_bf16(C, repeat=repeat, ph_a=ph_a, tail="narrow")


def build_program_fp8(C: int, repeat: int = 1, ph_a: int = 4):
    """Error-compensated fp8 DoubleRow program for row capacity C."""
    key = ("fp8", C, repeat, ph_a)
    if key in _nc_cache:
        return _nc_cache[key]
    assert C % P == 0
    m_tiles = C // P
    f32 = mybir.dt.float32
    f8 = mybir.dt.float8e4
    DR = mybir.MatmulPerfMode.DoubleRow
    MULT = mybir.AluOpType.mult
    ADD = mybir.AluOpType.add

    nc = bacc.Bacc(
        "TRN2", target_bir_lowering=False, debug=False, num_devices=N_CORES
    )
    # Blocked HBM layouts (prepared host-side) so every DMA moves large
    # contiguous per-partition runs:
    #   xT8/xTr[m, kp, ko, j]  = x8/r8[m*128+j, ko*128+kp]
    #   wT8/wTv[n, kp, ko, nn] = w8/v8^T[ko*128+kp, n*512+nn]
    xT8 = nc.dram_tensor("xT8", [m_tiles, P, KO, P], f8, kind="ExternalInput").ap()
    xTr = nc.dram_tensor("xTr", [m_tiles, P, KO, P], f8, kind="ExternalInput").ap()
    wT8 = nc.dram_tensor("wT8", [N_TILES, P, KO, N_TILE], f8, kind="ExternalInput").ap()
    wTv = nc.dram_tensor("wTv", [N_TILES, P, KO, N_TILE], f8, kind="ExternalInput").ap()
    bb = nc.dram_tensor("bb", [P, D_OUT], f32, kind="ExternalInput").ap()
    cc = nc.dram_tensor("cc", [P, 1], f32, kind="ExternalInput").ap()
    out = nc.dram_tensor("out", [C, D_OUT], f32, kind="ExternalOutput").ap()

    ph_a = min(ph_a, m_tiles)

    with tile.TileContext(nc) as tc:
        with (
            tc.tile_pool(name="wpool", bufs=1) as wpool,
            tc.tile_pool(name="cpool", bufs=1) as cpool,
            tc.tile_pool(name="xapool", bufs=1) as xapool,
            tc.tile_pool(name="xpool", bufs=3) as xpool,
            tc.tile_pool(name="opool", bufs=4) as opool,
            tc.tile_pool(name="ofpool", bufs=2) as ofpool,
            tc.tile_pool(name="pspool", bufs=8, space="PSUM") as pspool,
        ):
            w8_sb = wpool.tile([P, N_TILES, KO, N_TILE], f8, name="w8")
            v8_sb = wpool.tile([P, N_TILES, KO, N_TILE], f8, name="v8")
            b_sb = cpool.tile([P, D_OUT], f32, name="b")
            c_sb = cpool.tile([P, 1], f32, name="c")
            x8a = xapool.tile([P, ph_a, KO, P], f8, name="x8a")
            r8a = xapool.tile([P, ph_a, KO, P], f8, name="r8a")

            KQ = 4  # ko's per W chunk: 2 KB/partition, ~0.7 us transfer
            n_kq = KO // KQ

            def w_chunk(t_sb, t_hbm, n, kq):
                nc.sync.dma_start(
                    t_sb[:, n, kq * KQ : (kq + 1) * KQ],
                    t_hbm[n, :, kq * KQ : (kq + 1) * KQ],
                )

            # DMA issue order ~= HBM service order. The first x8 half-tile
            # and first w8 chunk lead so the first matmul's dependency chain
            # is short; per-(n,kq) w8/v8 chunks then interleave in exactly
            # the order phase A consumes them.
            nc.sync.dma_start(x8a[:, 0, : KO // 2], xT8[0, :, : KO // 2])
            nc.sync.dma_start(w8_sb[:, 0, :KQ], wT8[0, :, :KQ])
            nc.sync.dma_start(x8a[:, 0, KO // 2 :], xT8[0, :, KO // 2 :])
            nc.sync.dma_start(r8a[:, 0], xTr[0])
            w_chunk(v8_sb, wTv, 0, 0)
            if ph_a > 1:
                nc.sync.dma_start(x8a[:, 1], xT8[1])
                nc.sync.dma_start(r8a[:, 1], xTr[1])
            for kq in range(1, n_kq):
                w_chunk(w8_sb, wT8, 0, kq)
                w_chunk(v8_sb, wTv, 0, kq)
            for m in range(2, ph_a):
                nc.sync.dma_start(x8a[:, m], xT8[m])
                nc.sync.dma_start(r8a[:, m], xTr[m])
            nc.sync.dma_start(b_sb[:], bb[:])
            nc.sync.dma_start(c_sb[:], cc[:])
            for n in range(1, N_TILES):
                for kq in range(n_kq):
                    w_chunk(w8_sb, wT8, n, kq)
                    w_chunk(v8_sb, wTv, n, kq)

            def evict(ps, m, n, o_full=None):
                ms = slice(m * P, (m + 1) * P)
                ns = slice(n * N_TILE, (n + 1) * N_TILE)
                # out = psum * c + bias fused on the DVE
                if o_full is None:
                    o_sb = opool.tile([P, N_TILE], f32, tag="o")
                    nc.vector.scalar_tensor_tensor(
                        o_sb[:], ps, c_sb[:], b_sb[:, ns], MULT, ADD
                    )
                    # out DMAs ride the Activation HWDGE queue so their
                    # descriptor generation doesn't serialize behind the
                    # x/W loads on the SP queue
                    nc.scalar.dma_start(out[ms, ns], o_sb[:])
                else:
                    nc.vector.scalar_tensor_tensor(
                        o_full[:, ns], ps, c_sb[:], b_sb[:, ns], MULT, ADD
                    )

            # the three compensated products share one PSUM accumulation
            # group; per k-pair the product order matches W chunk delivery
            # (w8 then v8)
            def products(x8_t, r8_t):
                return ((x8_t, w8_sb), (x8_t, v8_sb), (r8_t, w8_sb))

            def mm_group(ps, x8_t, r8_t, n, q, t, start_q):
                xs, ws = products(x8_t, r8_t)[t]
                nc.tensor.matmul(
                    ps,
                    xs[:, 2 * q : 2 * q + 2],
                    ws[:, n, 2 * q : 2 * q + 2],
                    start=(q == start_q and t == 0),
                    stop=(q == NQ - 1 and t == 2),
                    perf_mode=DR,
                )

            def do_tile(x8_t, r8_t, m, n, o_full=None):
                ps = pspool.tile([P, N_TILE], f32, tag="ps")
                for q in range(NQ):
                    for t in range(3):
                        mm_group(ps, x8_t, r8_t, n, q, t, 0)
                evict(ps, m, n, o_full)

            def phase_a_quarter(n, m_set):
                pss = {}
                for m in m_set:
                    pss[m] = pspool.tile(
                        [P, N_TILE], f32, tag="ps", name=f"psA_{n}_{m}"
                    )
                for q in range(NQ):
                    for t in range(3):
                        for m in m_set:
                            mm_group(
                                pss[m], x8a[:, m], r8a[:, m], n, q, t, 0
                            )
                for m in m_set:
                    evict(pss[m], m, n)

            for rep in range(repeat):
                if rep == 0:
                    # phase A: k-major across resident x tiles per quarter.
                    # n=0 runs in two waves because only two x tiles have
                    # arrived when its chunks start landing.
                    first = list(range(min(2, ph_a)))
                    rest = list(range(min(2, ph_a), ph_a))
                    phase_a_quarter(0, first)
                    if rest:
                        phase_a_quarter(0, rest)
                    for n in range(1, N_TILES):
                        phase_a_quarter(n, list(range(ph_a)))
                    b_start = ph_a
                else:
                    b_start = 0
                # phase B: steady-state streaming; full-row out tiles so the
                # out DMA writes 8 KB/partition contiguous
                for m in range(b_start, m_tiles):
                    x8_sb = xpool.tile([P, KO, P], f8, tag="x8")
                    r8_sb = xpool.tile([P, KO, P], f8, tag="r8")
                    nc.sync.dma_start(x8_sb[:], xT8[m])
                    nc.sync.dma_start(r8_sb[:], xTr[m])
                    if m == m_tiles - 1:
                        # last tile: per-slice eviction so the final out DMA
                        # doesn't serialize behind all 4 evictions
                        for n in range(N_TILES):
                            do_tile(x8_sb, r8_sb, m, n)
                    else:
                        o_full = ofpool.tile([P, D_OUT], f32, tag="of")
                        for n in range(N_TILES):
                            do_tile(x8_sb, r8_sb, m, n, o_full=o_full)
                        nc.scalar.dma_start(
                            out[m * P : (m + 1) * P, :], o_full[:]
                        )

    nc.compile()
    _nc_cache[key] = nc
    return nc


def build_program_bf16(
    C: int,
    repeat: int = 1,
    ph_a: int = 4,
    inner: str = "n",
    out_dt: str = "f32",
    lead: str = "orig",
    tail: str = "orig",
):
    """bf16 program for row capacity C.

    inner: "n"  — per (m, n) psum group, n inner (16-matmul groups)
           "ko" — per m-tile, 4 psum groups interleaved ko-outer (stationary
                  x[:, ko] shared by 4 consecutive matmuls; group-boundary
                  sync amortized 4x)
           "ko2"— per 2 m-tiles, 8 psum groups interleaved ko-outer
    out_dt: "f32" | "bf16" — HBM dtype of out (bf16 halves store traffic;
           host casts back, ~2e-3 extra rounding)
    lead: "fine" — 2-ko first x piece + 1-ko W chunks on the act queue so
          the first matmul starts ~1.8 us earlier; first phase-A wave is
          m={0} alone so it only waits on x tile 0
    tail: "narrow" — last m-tile ends with 256/128/128-wide psum groups so
          the final evict+DMA after the last matmul is ~3x shorter
    """
    key = ("bf16", C, repeat, ph_a, inner, out_dt, lead, tail)
    if key in _nc_cache:
        return _nc_cache[key]
    assert C % P == 0
    m_tiles = C // P
    f32 = mybir.dt.float32
    bf16 = mybir.dt.bfloat16
    odt = f32 if out_dt == "f32" else bf16

    nc = bacc.Bacc(
        "TRN2", target_bir_lowering=False, debug=False, num_devices=N_CORES
    )
    xT = nc.dram_tensor("xT", [m_tiles, P, KO, P], bf16, kind="ExternalInput").ap()
    wT = nc.dram_tensor("wT", [N_TILES, P, KO, N_TILE], bf16, kind="ExternalInput").ap()
    bb = nc.dram_tensor("bb", [P, D_OUT], f32, kind="ExternalInput").ap()
    out = nc.dram_tensor("out", [C, D_OUT], odt, kind="ExternalOutput").ap()

    ph_a = min(ph_a, m_tiles)

    with tile.TileContext(nc) as tc:
        with (
            tc.tile_pool(name="wpool", bufs=1) as wpool,
            tc.tile_pool(name="cpool", bufs=1) as cpool,
            tc.tile_pool(name="xapool", bufs=1) as xapool,
            tc.tile_pool(name="xpool", bufs=3) as xpool,
            tc.tile_pool(name="opool", bufs=4) as opool,
            tc.tile_pool(name="ofpool", bufs=4) as ofpool,
            tc.tile_pool(name="pspool", bufs=8, space="PSUM") as pspool,
        ):
            w_sb = wpool.tile([P, N_TILES, KO, N_TILE], bf16)
            b_sb = cpool.tile([P, D_OUT], f32)
            xa_sb = xapool.tile([P, ph_a, KO, P], bf16)

            KQ = 4
            n_kq = KO // KQ

            def w_chunk(n, kq):
                nc.sync.dma_start(
                    w_sb[:, n, kq * KQ : (kq + 1) * KQ],
                    wT[n, :, kq * KQ : (kq + 1) * KQ],
                )

            def xa_dma(m):
                nc.sync.dma_start(xa_sb[:, m], xT[m])

            if lead == "par":
                # n=0 W rides the act queue in 2/2/4/8-ko pieces, in parallel
                # with x on the sync queue: the first matmul waits only
                # max(x0h1, w0[0:2]) ~= 2 us, and the m={0}-only first wave
                # consumes slowly enough (p-state ramp) to track delivery.
                nc.sync.dma_start(xa_sb[:, 0, : KO // 2], xT[0, :, : KO // 2])
                nc.scalar.dma_start(w_sb[:, 0, :2], wT[0, :, :2])
                nc.scalar.dma_start(w_sb[:, 0, 2:4], wT[0, :, 2:4])
                nc.scalar.dma_start(w_sb[:, 0, 4:8], wT[0, :, 4:8])
                nc.scalar.dma_start(w_sb[:, 0, 8:], wT[0, :, 8:])
                nc.sync.dma_start(xa_sb[:, 0, KO // 2 :], xT[0, :, KO // 2 :])
                for m in range(1, ph_a):
                    xa_dma(m)
                nc.sync.dma_start(b_sb[:], bb[:])
                for n in range(1, N_TILES):
                    for kq in range(n_kq):
                        w_chunk(n, kq)
            elif lead == "fine":
                # x tile 0 in 2/6/8-ko pieces on the sync queue; n=0 W in
                # 1/1/2/4/8-ko pieces on the act queue (parallel DGE) paced
                # just ahead of the ko-loop's consumption. First matmul only
                # waits ~1.8 us for x0[0:2] + w0[0:1].
                nc.sync.dma_start(xa_sb[:, 0, :2], xT[0, :, :2])
                nc.scalar.dma_start(w_sb[:, 0, :1], wT[0, :, :1])
                nc.sync.dma_start(xa_sb[:, 0, 2:8], xT[0, :, 2:8])
                nc.scalar.dma_start(w_sb[:, 0, 1:2], wT[0, :, 1:2])
                nc.sync.dma_start(xa_sb[:, 0, 8:], xT[0, :, 8:])
                nc.scalar.dma_start(w_sb[:, 0, 2:4], wT[0, :, 2:4])
                nc.scalar.dma_start(w_sb[:, 0, 4:8], wT[0, :, 4:8])
                nc.scalar.dma_start(w_sb[:, 0, 8:], wT[0, :, 8:])
                for m in range(1, ph_a):
                    xa_dma(m)
                nc.sync.dma_start(b_sb[:], bb[:])
                for n in range(1, N_TILES):
                    for kq in range(n_kq):
                        w_chunk(n, kq)
            else:
                nc.sync.dma_start(xa_sb[:, 0, : KO // 2], xT[0, :, : KO // 2])
                nc.sync.dma_start(w_sb[:, 0, :2], wT[0, :, :2])
                nc.sync.dma_start(xa_sb[:, 0, KO // 2 :], xT[0, :, KO // 2 :])
                nc.sync.dma_start(w_sb[:, 0, 2:4], wT[0, :, 2:4])
                if ph_a > 1:
                    xa_dma(1)
                for kq in range(1, n_kq):
                    w_chunk(0, kq)
                for m in range(2, ph_a):
                    xa_dma(m)
                nc.sync.dma_start(b_sb[:], bb[:])
                for n in range(1, N_TILES):
                    for kq in range(n_kq):
                        w_chunk(n, kq)

            def evict(ps, m, n):
                ms = slice(m * P, (m + 1) * P)
                ns = slice(n * N_TILE, (n + 1) * N_TILE)
                o_sb = opool.tile([P, N_TILE], odt, tag="o", name=f"o_{m}_{n}")
                nc.vector.tensor_add(o_sb[:], ps, b_sb[:, ns])
                nc.scalar.dma_start(out[ms, ns], o_sb[:])

            def do_group(x_tile, m, n, o_full=None):
                ps = pspool.tile([P, N_TILE], f32, tag="ps")
                for ko in range(KO):
                    nc.tensor.matmul(
                        ps,
                        x_tile[:, ko],
                        w_sb[:, n, ko],
                        start=(ko == 0),
                        stop=(ko == KO - 1),
                    )
                if o_full is None:
                    evict(ps, m, n)
                else:
                    ns = slice(n * N_TILE, (n + 1) * N_TILE)
                    nc.vector.tensor_add(o_full[:, ns], ps, b_sb[:, ns])

            def do_group_cols(x_tile, m, c0, c1):
                """Narrow trailing psum group over out cols [c0, c1) (within
                one n-tile); psum/o_sb are slices of full-width pool tiles."""
                w = c1 - c0
                n = c0 // N_TILE
                ps_full = pspool.tile(
                    [P, N_TILE], f32, tag="ps", name=f"psn_{m}_{c0}"
                )
                ps = ps_full[:, :w]
                for ko in range(KO):
                    nc.tensor.matmul(
                        ps,
                        x_tile[:, ko],
                        w_sb[:, n, ko, c0 - n * N_TILE : c1 - n * N_TILE],
                        start=(ko == 0),
                        stop=(ko == KO - 1),
                    )
                o_sb = opool.tile(
                    [P, N_TILE], odt, tag="o", name=f"on_{m}_{c0}"
                )
                nc.vector.tensor_add(o_sb[:, :w], ps, b_sb[:, c0:c1])
                nc.scalar.dma_start(
                    out[m * P : (m + 1) * P, c0:c1], o_sb[:, :w]
                )

            def do_last_tile(x_tile, m):
                if tail == "narrow":
                    for n in range(N_TILES - 1):
                        do_group(x_tile, m, n)
                    base = (N_TILES - 1) * N_TILE
                    for c0, c1 in (
                        (base, base + 256),
                        (base + 256, base + 384),
                        (base + 384, base + 512),
                    ):
                        do_group_cols(x_tile, m, c0, c1)
                else:
                    for n in range(N_TILES):
                        do_group(x_tile, m, n)

            def do_mtiles_ko_outer(x_tiles, ms, o_fulls):
                """Interleave len(ms)*N_TILES psum groups ko-outer; 4
                consecutive matmuls share the stationary x[:, ko]."""
                pss = {}
                for m in ms:
                    for n in range(N_TILES):
                        pss[m, n] = pspool.tile(
                            [P, N_TILE], f32, tag="ps", name=f"ps_{m}_{n}"
                        )
                for ko in range(KO):
                    for mi, m in enumerate(ms):
                        for n in range(N_TILES):
                            nc.tensor.matmul(
                                pss[m, n],
                                x_tiles[mi][:, ko],
                                w_sb[:, n, ko],
                                start=(ko == 0),
                                stop=(ko == KO - 1),
                            )
                for mi, m in enumerate(ms):
                    for n in range(N_TILES):
                        ns = slice(n * N_TILE, (n + 1) * N_TILE)
                        if o_fulls[mi] is None:
                            evict(pss[m, n], m, n)
                        else:
                            nc.vector.tensor_add(
                                o_fulls[mi][:, ns], pss[m, n], b_sb[:, ns]
                            )

            def phase_a_quarter(n, m_set):
                pss = {}
                for m in m_set:
                    pss[m] = pspool.tile(
                        [P, N_TILE], f32, tag="ps", name=f"psA_{n}_{m}"
                    )
                for ko in range(KO):
                    for m in m_set:
                        nc.tensor.matmul(
                            pss[m],
                            xa_sb[:, m, ko],
                            w_sb[:, n, ko],
                            start=(ko == 0),
                            stop=(ko == KO - 1),
                        )
                for m in m_set:
                    evict(pss[m], m, n)

            for rep in range(repeat):
                if rep == 0:
                    n_first = 1 if lead in ("fine", "par") else min(2, ph_a)
                    first = list(range(n_first))
                    rest = list(range(n_first, ph_a))
                    phase_a_quarter(0, first)
                    if rest:
                        phase_a_quarter(0, rest)
                    for n in range(1, N_TILES):
                        phase_a_quarter(n, list(range(ph_a)))
                    b_start = ph_a
                else:
                    b_start = 0
                if inner == "n":
                    for m in range(b_start, m_tiles):
                        x_sb = xpool.tile([P, KO, P], bf16, tag="x")
                        nc.sync.dma_start(x_sb[:], xT[m])
                        if m == m_tiles - 1:
                            do_last_tile(x_sb, m)
                        else:
                            o_full = ofpool.tile([P, D_OUT], odt, tag="of")
                            for n in range(N_TILES):
                                do_group(x_sb, m, n, o_full=o_full)
                            nc.scalar.dma_start(
                                out[m * P : (m + 1) * P, :], o_full[:]
                            )
                else:
                    step = 2 if inner == "ko2" else 1
                    m = b_start
                    while m < m_tiles:
                        ms = list(range(m, min(m + step, m_tiles)))
                        x_tiles, o_fulls = [], []
                        for mi in ms:
                            x_sb = xpool.tile(
                                [P, KO, P], bf16, tag="x", name=f"x_{mi}"
                            )
                            nc.sync.dma_start(x_sb[:], xT[mi])
                            x_tiles.append(x_sb)
                            if mi == m_tiles - 1:
                                o_fulls.append(None)
                            else:
                                of_sb = ofpool.tile(
                                    [P, D_OUT], odt, tag="of", name=f"of_{mi}"
                                )
                                o_fulls.append(of_sb)
                        do_mtiles_ko_outer(x_tiles, ms, o_fulls)
                        for mi, of in zip(ms, o_fulls):
                            if of is not None:
                                nc.scalar.dma_start(
                                    out[mi * P : (mi + 1) * P, :], of[:]
                                )
                        m += step

    nc.compile()
    _nc_cache[key] = nc
    return nc


NB = 256  # borrow half-unit width (cols)


def build_program_bal(
    m_main: int, bh: int, repeat: int = 1, ph_a: int = 4, lead2: bool = False,
    no_out: bool = False,
):
    """Balanced program: m_main own-expert m-tiles (full D_OUT) plus bh
    borrowed half-units (one x-tile x NB cols each, self-contained W/bias
    slices) redistributing overflow tiles of heavy experts. Perfectly
    balances total padded tile work across the 8 cores.
    """
    key = ("bal", m_main, bh, repeat, ph_a, lead2, no_out)
    if key in _nc_cache:
        return _nc_cache[key]
    f32 = mybir.dt.float32
    bf16 = mybir.dt.bfloat16
    C = m_main * P

    nc = bacc.Bacc(
        "TRN2", target_bir_lowering=False, debug=False, num_devices=N_CORES
    )
    xT = nc.dram_tensor("xT", [m_main, P, KO, P], bf16, kind="ExternalInput").ap()
    wT = nc.dram_tensor("wT", [N_TILES, P, KO, N_TILE], bf16, kind="ExternalInput").ap()
    bb = nc.dram_tensor("bb", [P, D_OUT], f32, kind="ExternalInput").ap()
    xB = nc.dram_tensor("xB", [max(bh, 1), P, KO, P], bf16, kind="ExternalInput").ap()
    wB = nc.dram_tensor("wB", [max(bh, 1), P, KO, NB], bf16, kind="ExternalInput").ap()
    bB = nc.dram_tensor("bB", [P, max(bh, 1), NB], f32, kind="ExternalInput").ap()
    out = nc.dram_tensor("out", [C, D_OUT], f32, kind="ExternalOutput").ap()
    outB = nc.dram_tensor(
        "outB", [max(bh, 1) * P, NB], f32, kind="ExternalOutput"
    ).ap()

    ph_a = min(ph_a, m_main)

    with tile.TileContext(nc) as tc:
        with (
            tc.tile_pool(name="wpool", bufs=1) as wpool,
            tc.tile_pool(name="cpool", bufs=1) as cpool,
            tc.tile_pool(name="xapool", bufs=1) as xapool,
            tc.tile_pool(name="xpool", bufs=3) as xpool,
            tc.tile_pool(name="xbpool", bufs=4) as xbpool,
            tc.tile_pool(name="opool", bufs=4) as opool,
            tc.tile_pool(name="ofpool", bufs=3) as ofpool,
            tc.tile_pool(name="pspool", bufs=8, space="PSUM") as pspool,
        ):
            w_sb = wpool.tile([P, N_TILES, KO, N_TILE], bf16, name="w")
            wb_sb = wpool.tile([P, max(bh, 1), KO, NB], bf16, name="wb")
            b_sb = cpool.tile([P, D_OUT], f32, name="b")
            bb_sb = cpool.tile([P, max(bh, 1), NB], f32, name="bb2")
            xa_sb = xapool.tile([P, ph_a, KO, P], bf16, name="xa")

            KQ = 4
            n_kq = KO // KQ

            def w_chunk(n, kq):
                nc.sync.dma_start(
                    w_sb[:, n, kq * KQ : (kq + 1) * KQ],
                    wT[n, :, kq * KQ : (kq + 1) * KQ],
                )

            def xa_dma(m):
                nc.sync.dma_start(xa_sb[:, m], xT[m])

            wq = nc.scalar if lead2 else nc.sync
            nc.sync.dma_start(xa_sb[:, 0, : KO // 2], xT[0, :, : KO // 2])
            wq.dma_start(w_sb[:, 0, :2], wT[0, :, :2])
            nc.sync.dma_start(xa_sb[:, 0, KO // 2 :], xT[0, :, KO // 2 :])
            wq.dma_start(w_sb[:, 0, 2:4], wT[0, :, 2:4])
            if ph_a > 1:
                xa_dma(1)
            for kq in range(1, n_kq):
                (wq if lead2 else nc.sync).dma_start(
                    w_sb[:, 0, kq * KQ : (kq + 1) * KQ],
                    wT[0, :, kq * KQ : (kq + 1) * KQ],
                )
            for m in range(2, ph_a):
                xa_dma(m)
            nc.sync.dma_start(b_sb[:], bb[:])
            for n in range(1, N_TILES):
                for kq in range(n_kq):
                    w_chunk(n, kq)
            for u in range(bh):
                nc.sync.dma_start(wb_sb[:, u], wB[u])
            nc.sync.dma_start(bb_sb[:], bB[:])

            def evict(ps, m, n):
                if no_out:
                    return
                ms = slice(m * P, (m + 1) * P)
                ns = slice(n * N_TILE, (n + 1) * N_TILE)
                o_sb = opool.tile([P, N_TILE], f32, tag="o", name=f"o_{m}_{n}")
                nc.vector.tensor_add(o_sb[:], ps, b_sb[:, ns])
                nc.scalar.dma_start(out[ms, ns], o_sb[:])

            def do_group(x_tile, m, n, o_full=None):
                ps = pspool.tile([P, N_TILE], f32, tag="ps")
                for ko in range(KO):
                    nc.tensor.matmul(
                        ps,
                        x_tile[:, ko],
                        w_sb[:, n, ko],
                        start=(ko == 0),
                        stop=(ko == KO - 1),
                    )
                if no_out:
                    pass
                elif o_full is None:
                    evict(ps, m, n)
                else:
                    ns = slice(n * N_TILE, (n + 1) * N_TILE)
                    nc.vector.tensor_add(o_full[:, ns], ps, b_sb[:, ns])

            def do_borrow(u):
                x_sb = xbpool.tile([P, KO, P], bf16, tag="xb", name=f"xb_{u}")
                nc.sync.dma_start(x_sb[:], xB[u])
                ps_full = pspool.tile([P, N_TILE], f32, tag="ps", name=f"psb_{u}")
                ps = ps_full[:, :NB]
                for ko in range(KO):
                    nc.tensor.matmul(
                        ps,
                        x_sb[:, ko],
                        wb_sb[:, u, ko],
                        start=(ko == 0),
                        stop=(ko == KO - 1),
                    )
                if no_out:
                    return
                o_sb = opool.tile([P, N_TILE], f32, tag="o", name=f"ob_{u}")
                nc.vector.tensor_add(o_sb[:, :NB], ps, bb_sb[:, u])
                nc.scalar.dma_start(outB[u * P : (u + 1) * P, :], o_sb[:, :NB])

            def phase_a_quarter(n, m_set):
                pss = {}
                for m in m_set:
                    pss[m] = pspool.tile(
                        [P, N_TILE], f32, tag="ps", name=f"psA_{n}_{m}"
                    )
                for ko in range(KO):
                    for m in m_set:
                        nc.tensor.matmul(
                            pss[m],
                            xa_sb[:, m, ko],
                            w_sb[:, n, ko],
                            start=(ko == 0),
                            stop=(ko == KO - 1),
                        )
                for m in m_set:
                    evict(pss[m], m, n)

            for rep in range(repeat):
                if rep == 0:
                    first = list(range(min(2, ph_a)))
                    rest = list(range(min(2, ph_a), ph_a))
                    phase_a_quarter(0, first)
                    if rest:
                        phase_a_quarter(0, rest)
                    for n in range(1, N_TILES):
                        phase_a_quarter(n, list(range(ph_a)))
                    b_start = ph_a
                else:
                    b_start = 0
                for m in range(b_start, m_main):
                    x_sb = xpool.tile([P, KO, P], bf16, tag="x")
                    nc.sync.dma_start(x_sb[:], xT[m])
                    if no_out:
                        for n in range(N_TILES):
                            do_group(x_sb, m, n)
                        continue
                    o_full = ofpool.tile([P, D_OUT], f32, tag="of")
                    for n in range(N_TILES):
                        do_group(x_sb, m, n, o_full=o_full)
                    nc.scalar.dma_start(out[m * P : (m + 1) * P, :], o_full[:])
                # borrowed half-units last: short 256-col groups keep the
                # post-last-matmul evict+DMA tail small
                for u in range(bh):
                    do_borrow(u)

    nc.compile()
    _nc_cache[key] = nc
    return nc


def plan_bal(counts):
    """Balanced assignment. Returns (m_main, bh, units) where units[c] is
    the per-core list of (g, mt, n2) borrowed half-units (None = dummy)."""
    tiles = [int(math.ceil(c / 128)) if c else 0 for c in counts]
    m_main = max(1, min(t for t in tiles))
    pool = []
    for g in range(NUM_GROUPS):
        for mt in range(m_main, tiles[g]):
            for n2 in range(D_OUT // NB):
                pool.append((g, mt, n2))
    bh = (len(pool) + N_CORES - 1) // N_CORES
    units = []
    for c in range(N_CORES):
        us = pool[c * bh : (c + 1) * bh]
        us += [None] * (bh - len(us))
        units.append(us)
    return m_main, bh, units


BAL_PLAN = None


def shard_inputs(x, weight, bias, group_indices, mode: str | None = None):
    """Host-side expert-parallel sharding. Returns (in_maps, perm, offsets,
    counts, C)."""
    mode = MODE if mode is None else mode
    n_rows = x.shape[0]
    gi = np.asarray(group_indices)
    perm = np.argsort(gi, kind="stable")
    counts = np.bincount(gi, minlength=NUM_GROUPS).astype(np.int64)
    offsets = np.zeros(NUM_GROUPS + 1, dtype=np.int64)
    np.cumsum(counts, out=offsets[1:])
    C = max(P, int(math.ceil(counts.max() / P)) * P)

    x_sorted = x[perm] if not np.array_equal(perm, np.arange(n_rows)) else x
    m_tiles = C // P

    def block_x(xg):
        # [C, D_IN] -> [m, kp, ko, j] with xT[m, kp, ko, j] = xg[m*128+j, ko*128+kp]
        return np.ascontiguousarray(
            xg.reshape(m_tiles, P, KO, P).transpose(0, 3, 2, 1)
        )

    def block_w(wg):
        # W^T [D_IN, D_OUT] -> [n, kp, ko, nn]
        return np.ascontiguousarray(
            wg.T.reshape(KO, P, N_TILES, N_TILE).transpose(2, 1, 0, 3)
        )

    in_maps = []
    if mode == "bal":
        global BAL_PLAN
        m_main, bh, units = plan_bal(counts)
        BAL_PLAN = (m_main, bh, units)
        Cm = m_main * P

        def block_xg(xg, mt):
            return np.ascontiguousarray(
                xg.reshape(mt, P, KO, P).transpose(0, 3, 2, 1)
            ).astype(BF16)

        for c in range(N_CORES):
            g = c
            ng = int(min(counts[g], Cm))
            xg = np.zeros((Cm, D_IN), dtype=np.float32)
            xg[:ng] = x_sorted[offsets[g] : offsets[g] + ng]
            xBa = np.zeros((max(bh, 1), P, KO, P), dtype=BF16)
            wBa = np.zeros((max(bh, 1), P, KO, NB), dtype=BF16)
            bBa = np.zeros((P, max(bh, 1), NB), dtype=np.float32)
            for u, unit in enumerate(units[c]):
                if unit is None:
                    continue
                gu, mt, n2 = unit
                r0 = mt * P
                nr = int(min(P, counts[gu] - r0))
                xu = np.zeros((P, D_IN), dtype=np.float32)
                xu[:nr] = x_sorted[offsets[gu] + r0 : offsets[gu] + r0 + nr]
                xBa[u] = block_xg(xu, 1)[0]
                wslice = weight[gu].T[:, n2 * NB : (n2 + 1) * NB]
                wBa[u] = np.ascontiguousarray(
                    wslice.reshape(KO, P, NB).transpose(1, 0, 2)
                ).astype(BF16)
                bBa[:, u, :] = bias[gu][n2 * NB : (n2 + 1) * NB]
            in_maps.append(
                {
                    "xT": block_xg(xg, m_main),
                    "wT": block_w(weight[g]).astype(BF16),
                    "bb": np.ascontiguousarray(
                        np.broadcast_to(bias[g], (P, D_OUT))
                    ),
                    "xB": xBa,
                    "wB": wBa,
                    "bB": bBa,
                }
            )
        return in_maps, perm, offsets, counts, C
    if mode == "fp8":
        sx = np.float32(120.0) / max(np.abs(x).max(), 1e-30)
        sw = np.float32(120.0) / max(np.abs(weight).max(), 1e-30)
        c = np.float32(1.0) / (np.float32(sx) * np.float32(sw))
        cc = np.full((P, 1), c, dtype=np.float32)
        for g in range(NUM_GROUPS):
            ng = int(counts[g])
            xg = np.zeros((C, D_IN), dtype=np.float32)
            xg[:ng] = x_sorted[offsets[g] : offsets[g] + ng]
            xs = xg * sx
            x8 = xs.astype(F8)
            r8 = (xs - x8.astype(np.float32)).astype(F8)
            ws = weight[g] * sw
            w8 = ws.astype(F8)
            v8 = (ws - w8.astype(np.float32)).astype(F8)
            in_maps.append(
                {
                    "xT8": block_x(x8),
                    "xTr": block_x(r8),
                    "wT8": block_w(w8),
                    "wTv": block_w(v8),
                    "bb": np.ascontiguousarray(
                        np.broadcast_to(bias[g], (P, D_OUT))
                    ),
                    "cc": cc,
                }
            )
    else:
        for g in range(NUM_GROUPS):
            ng = int(counts[g])
            xg = np.zeros((C, D_IN), dtype=np.float32)
            xg[:ng] = x_sorted[offsets[g] : offsets[g] + ng]
            in_maps.append(
                {
                    "xT": block_x(xg).astype(BF16),
                    "wT": block_w(weight[g]).astype(BF16),
                    "bb": np.ascontiguousarray(
                        np.broadcast_to(bias[g], (P, D_OUT))
                    ),
                }
            )
    return in_maps, perm, offsets, counts, C


def unshard_output(results, perm, offsets, counts, n_rows):
    out = np.empty((n_rows, D_OUT), dtype=np.float32)
    for g in range(NUM_GROUPS):
        ng = int(counts[g])
        out[perm[offsets[g] : offsets[g] + ng]] = results[g]["out"][:ng]
    return out


def unshard_output_bal(results, perm, offsets, counts, n_rows):
    m_main, bh, units = BAL_PLAN
    Cm = m_main * P
    out = np.empty((n_rows, D_OUT), dtype=np.float32)
    for g in range(NUM_GROUPS):
        ng = int(min(counts[g], Cm))
        out[perm[offsets[g] : offsets[g] + ng]] = results[g]["out"][:ng]
    for c in range(N_CORES):
        for u, unit in enumerate(units[c]):
            if unit is None:
                continue
            gu, mt, n2 = unit
            r0 = mt * P
            nr = int(min(P, counts[gu] - r0))
            rows = perm[offsets[gu] + r0 : offsets[gu] + r0 + nr]
            out[rows, n2 * NB : (n2 + 1) * NB] = results[c]["outB"][
                u * P : u * P + nr
            ]
    return out


def kernel(x, weight, bias, group_indices):
    x = np.asarray(x, dtype=np.float32)
    weight = np.asarray(weight, dtype=np.float32)
    bias = np.asarray(bias, dtype=np.float32)
    group_indices = np.asarray(group_indices)
    assert x.shape[1] == D_IN and weight.shape == (NUM_GROUPS, D_OUT, D_IN)

    mode = MODE
    if mode == "bal":
        # fall back to the plain expert-parallel program when the group
        # distribution is too skewed for the borrow plan (keeps SBUF for
        # borrowed W slices bounded)
        gi = np.asarray(group_indices)
        cts = np.bincount(gi, minlength=NUM_GROUPS)
        m_main, bh, _units = plan_bal(cts)
        if bh > 8 or m_main < 4:
            mode = "bf16"
    in_maps, perm, offsets, counts, C = shard_inputs(
        x, weight, bias, group_indices, mode=mode
    )
    nc = build_program(C, mode=mode)
    res = run_bass_kernel_spmd(nc, in_maps, core_ids=list(range(N_CORES)))
    if mode == "bal":
        return unshard_output_bal(res.results, perm, offsets, counts, x.shape[0])
    return unshard_output(res.results, perm, offsets, counts, x.shape[0])


def _sim_main():
    import tsim

    C = 2176
    for mode in ("fp8", "bf16"):
        _nc_cache.clear()
        nc = build_program(C, repeat=1, mode=mode)
        t1, _ = tsim.simulate(nc)
        nc3 = build_program(C, repeat=3, mode=mode)
        t3, _ = tsim.simulate(nc3)
        body = (t3 - t1) / 2
        print(
            f"{mode}: full {t1 / 1e3:.1f} us, body {body / 1e3:.1f} us,"
            f" overhead {(t1 - body) / 1e3:.1f} us"
        )


if __name__ == "__main__":
    _sim_main()


# revision 39
# speedup vs baseline: 1.1102x; 1.0145x over previous
"""Grouped linear (MoE routing) kernel for 8 Trainium2 NeuronCores.

out[n] = x[n] @ weight[g[n]].T + bias[g[n]]

Strategy: expert-parallel with overflow balancing (MODE="bal").
group_indices is (assumed) sorted; host code computes per-group row
ranges. Core g computes the dense bf16 GEMM for expert g's first
m_main=min_g(tiles_g) 128-row tiles (x and W cast to bf16, fp32 PSUM +
fp32 bias/output, ~2.4e-3 rel err), entirely on-core with no
collectives. Each overflow tile of heavier experts (tiles beyond
m_main) is split into 8 x 256-col units, one per core, so every core's
borrow slots have identical row counts (SPMD-uniform program). The
borrowed matmuls use the tile's ROWS as the moving dim (W col-slice
stationary, transposed [n, rows] psum/output, per-partition bias), so a
partial overflow tile costs only its real rows (32*r cycles, not a
full padded tile's 4096) — per-core PE floor drops from
ceil(max_g)/128 tiles (232.1 us at C=2176) to 220.3 us. Host
gathers/scatters rows, including transposing borrowed-unit outputs.

Schedule per core: W stays resident in SBUF, DMA'd in per-(n, 4-ko)
chunks interleaved with the phase-A x tiles so the first matmul waits
only ~3.5 us and the PE tracks delivery; phase B streams the remaining
m-tiles (full-row fp32 out tiles, 8 KB/partition DMAs); the borrowed
units run two tiles into phase B (interleaved across <=8 psum banks,
after phase A's eviction backlog drains) so their tiny evict+DMA
chains hide under main compute; the last main tile ends with narrowing
512/512/512/256/128/128-col groups to keep the drain tail short.

Measured on HW (chain-differencing, see test.py): steady body ~228 us
vs the 220.3 us floor; reported full-kernel 239.1 us vs 259 us for the
previous C=2176 unbalanced baseline (~8% faster). The remaining ~8 us
body-vs-floor gap is unmodeled HW overhead (TimelineSim puts the body
exactly at the floor); ko-outer interleaving, bf16 stores, and lead-in
reshuffles were all measured and do not close it.

Dead ends measured on this HW (kept as modes for reference):
  - fp8 DoubleRow (MODE="fp8"): the cost model's 0.5 cycles/row is
    wrong on HW — a DoubleRow instruction costs the same 512 cycles as
    a bf16 one (2x FLOPs via 2 folded k-subtiles, not 4x). The
    error-compensated 3-product split (x8@w8 + x8@v8 + r8@w8, rel err
    1.1e-3) therefore runs 1.5x SLOWER than bf16 (349 us measured vs
    348.2 predicted at 1.0 c/r). Pure/2-term fp8 is fast enough but
    fails the 2e-2 gate (3.7% / 2.6% rel err).
  - inner="ko"/"ko2" interleaved-psum schedules: no gain (236.7) /
    worse (245.1) vs inner="n" (235.1) — group-boundary sync is already
    hidden; 8-bank-deep interleave adds psum-reuse bubbles.
  - bf16 out store: no gain (DMA is not the bottleneck at 36 MB/core).
  - finer/parallel-queue lead-ins: the original x-half + W-chunk lead
    with a {0,1} first wave is already optimal; starting earlier starves
    the ko loop on W delivery instead.
"""

import math
import sys

for _p in ("/opt/trn_rl_repo", "/root/.axon_site/_ro/trn_rl_repo"):
    if _p not in sys.path:
        sys.path.append(_p)

import ml_dtypes
import numpy as np

BF16 = np.dtype(ml_dtypes.bfloat16)
F8 = np.dtype(ml_dtypes.float8_e4m3)

from concourse import bacc, mybir, tile
from concourse.bass_utils import run_bass_kernel_spmd

P = 128
D_IN = 2048
D_OUT = 2048
KO = D_IN // P  # 16 k-subtiles
NQ = KO // 2  # 8 DoubleRow k-pair groups
N_TILE = 512
N_TILES = D_OUT // N_TILE  # 4
NUM_GROUPS = 8
N_CORES = 8

MODE = "bal"

_nc_cache: dict = {}


def build_program(C: int, repeat: int = 1, ph_a: int = 4, mode: str | None = None):
    mode = MODE if mode is None else mode
    if mode == "fp8":
        return build_program_fp8(C, repeat=repeat, ph_a=ph_a)
    if mode == "bal":
        m_main, bh, _units = BAL_PLAN
        return build_program_bal(m_main, bh, repeat=repeat, ph_a=ph_a)
    return build_program_bf16(C, repeat=repeat, ph_a=ph_a, tail="narrow")


def build_program_fp8(C: int, repeat: int = 1, ph_a: int = 4):
    """Error-compensated fp8 DoubleRow program for row capacity C."""
    key = ("fp8", C, repeat, ph_a)
    if key in _nc_cache:
        return _nc_cache[key]
    assert C % P == 0
    m_tiles = C // P
    f32 = mybir.dt.float32
    f8 = mybir.dt.float8e4
    DR = mybir.MatmulPerfMode.DoubleRow
    MULT = mybir.AluOpType.mult
    ADD = mybir.AluOpType.add

    nc = bacc.Bacc(
        "TRN2", target_bir_lowering=False, debug=False, num_devices=N_CORES
    )
    # Blocked HBM layouts (prepared host-side) so every DMA moves large
    # contiguous per-partition runs:
    #   xT8/xTr[m, kp, ko, j]  = x8/r8[m*128+j, ko*128+kp]
    #   wT8/wTv[n, kp, ko, nn] = w8/v8^T[ko*128+kp, n*512+nn]
    xT8 = nc.dram_tensor("xT8", [m_tiles, P, KO, P], f8, kind="ExternalInput").ap()
    xTr = nc.dram_tensor("xTr", [m_tiles, P, KO, P], f8, kind="ExternalInput").ap()
    wT8 = nc.dram_tensor("wT8", [N_TILES, P, KO, N_TILE], f8, kind="ExternalInput").ap()
    wTv = nc.dram_tensor("wTv", [N_TILES, P, KO, N_TILE], f8, kind="ExternalInput").ap()
    bb = nc.dram_tensor("bb", [P, D_OUT], f32, kind="ExternalInput").ap()
    cc = nc.dram_tensor("cc", [P, 1], f32, kind="ExternalInput").ap()
    out = nc.dram_tensor("out", [C, D_OUT], f32, kind="ExternalOutput").ap()

    ph_a = min(ph_a, m_tiles)

    with tile.TileContext(nc) as tc:
        with (
            tc.tile_pool(name="wpool", bufs=1) as wpool,
            tc.tile_pool(name="cpool", bufs=1) as cpool,
            tc.tile_pool(name="xapool", bufs=1) as xapool,
            tc.tile_pool(name="xpool", bufs=3) as xpool,
            tc.tile_pool(name="opool", bufs=4) as opool,
            tc.tile_pool(name="ofpool", bufs=2) as ofpool,
            tc.tile_pool(name="pspool", bufs=8, space="PSUM") as pspool,
        ):
            w8_sb = wpool.tile([P, N_TILES, KO, N_TILE], f8, name="w8")
            v8_sb = wpool.tile([P, N_TILES, KO, N_TILE], f8, name="v8")
            b_sb = cpool.tile([P, D_OUT], f32, name="b")
            c_sb = cpool.tile([P, 1], f32, name="c")
            x8a = xapool.tile([P, ph_a, KO, P], f8, name="x8a")
            r8a = xapool.tile([P, ph_a, KO, P], f8, name="r8a")

            KQ = 4  # ko's per W chunk: 2 KB/partition, ~0.7 us transfer
            n_kq = KO // KQ

            def w_chunk(t_sb, t_hbm, n, kq):
                nc.sync.dma_start(
                    t_sb[:, n, kq * KQ : (kq + 1) * KQ],
                    t_hbm[n, :, kq * KQ : (kq + 1) * KQ],
                )

            # DMA issue order ~= HBM service order. The first x8 half-tile
            # and first w8 chunk lead so the first matmul's dependency chain
            # is short; per-(n,kq) w8/v8 chunks then interleave in exactly
            # the order phase A consumes them.
            nc.sync.dma_start(x8a[:, 0, : KO // 2], xT8[0, :, : KO // 2])
            nc.sync.dma_start(w8_sb[:, 0, :KQ], wT8[0, :, :KQ])
            nc.sync.dma_start(x8a[:, 0, KO // 2 :], xT8[0, :, KO // 2 :])
            nc.sync.dma_start(r8a[:, 0], xTr[0])
            w_chunk(v8_sb, wTv, 0, 0)
            if ph_a > 1:
                nc.sync.dma_start(x8a[:, 1], xT8[1])
                nc.sync.dma_start(r8a[:, 1], xTr[1])
            for kq in range(1, n_kq):
                w_chunk(w8_sb, wT8, 0, kq)
                w_chunk(v8_sb, wTv, 0, kq)
            for m in range(2, ph_a):
                nc.sync.dma_start(x8a[:, m], xT8[m])
                nc.sync.dma_start(r8a[:, m], xTr[m])
            nc.sync.dma_start(b_sb[:], bb[:])
            nc.sync.dma_start(c_sb[:], cc[:])
            for n in range(1, N_TILES):
                for kq in range(n_kq):
                    w_chunk(w8_sb, wT8, n, kq)
                    w_chunk(v8_sb, wTv, n, kq)

            def evict(ps, m, n, o_full=None):
                ms = slice(m * P, (m + 1) * P)
                ns = slice(n * N_TILE, (n + 1) * N_TILE)
                # out = psum * c + bias fused on the DVE
                if o_full is None:
                    o_sb = opool.tile([P, N_TILE], f32, tag="o")
                    nc.vector.scalar_tensor_tensor(
                        o_sb[:], ps, c_sb[:], b_sb[:, ns], MULT, ADD
                    )
                    # out DMAs ride the Activation HWDGE queue so their
                    # descriptor generation doesn't serialize behind the
                    # x/W loads on the SP queue
                    nc.scalar.dma_start(out[ms, ns], o_sb[:])
                else:
                    nc.vector.scalar_tensor_tensor(
                        o_full[:, ns], ps, c_sb[:], b_sb[:, ns], MULT, ADD
                    )

            # the three compensated products share one PSUM accumulation
            # group; per k-pair the product order matches W chunk delivery
            # (w8 then v8)
            def products(x8_t, r8_t):
                return ((x8_t, w8_sb), (x8_t, v8_sb), (r8_t, w8_sb))

            def mm_group(ps, x8_t, r8_t, n, q, t, start_q):
                xs, ws = products(x8_t, r8_t)[t]
                nc.tensor.matmul(
                    ps,
                    xs[:, 2 * q : 2 * q + 2],
                    ws[:, n, 2 * q : 2 * q + 2],
                    start=(q == start_q and t == 0),
                    stop=(q == NQ - 1 and t == 2),
                    perf_mode=DR,
                )

            def do_tile(x8_t, r8_t, m, n, o_full=None):
                ps = pspool.tile([P, N_TILE], f32, tag="ps")
                for q in range(NQ):
                    for t in range(3):
                        mm_group(ps, x8_t, r8_t, n, q, t, 0)
                evict(ps, m, n, o_full)

            def phase_a_quarter(n, m_set):
                pss = {}
                for m in m_set:
                    pss[m] = pspool.tile(
                        [P, N_TILE], f32, tag="ps", name=f"psA_{n}_{m}"
                    )
                for q in range(NQ):
                    for t in range(3):
                        for m in m_set:
                            mm_group(
                                pss[m], x8a[:, m], r8a[:, m], n, q, t, 0
                            )
                for m in m_set:
                    evict(pss[m], m, n)

            for rep in range(repeat):
                if rep == 0:
                    # phase A: k-major across resident x tiles per quarter.
                    # n=0 runs in two waves because only two x tiles have
                    # arrived when its chunks start landing.
                    first = list(range(min(2, ph_a)))
                    rest = list(range(min(2, ph_a), ph_a))
                    phase_a_quarter(0, first)
                    if rest:
                        phase_a_quarter(0, rest)
                    for n in range(1, N_TILES):
                        phase_a_quarter(n, list(range(ph_a)))
                    b_start = ph_a
                else:
                    b_start = 0
                # phase B: steady-state streaming; full-row out tiles so the
                # out DMA writes 8 KB/partition contiguous
                for m in range(b_start, m_tiles):
                    x8_sb = xpool.tile([P, KO, P], f8, tag="x8")
                    r8_sb = xpool.tile([P, KO, P], f8, tag="r8")
                    nc.sync.dma_start(x8_sb[:], xT8[m])
                    nc.sync.dma_start(r8_sb[:], xTr[m])
                    if m == m_tiles - 1:
                        # last tile: per-slice eviction so the final out DMA
                        # doesn't serialize behind all 4 evictions
                        for n in range(N_TILES):
                            do_tile(x8_sb, r8_sb, m, n)
                    else:
                        o_full = ofpool.tile([P, D_OUT], f32, tag="of")
                        for n in range(N_TILES):
                            do_tile(x8_sb, r8_sb, m, n, o_full=o_full)
                        nc.scalar.dma_start(
                            out[m * P : (m + 1) * P, :], o_full[:]
                        )

    nc.compile()
    _nc_cache[key] = nc
    return nc


def build_program_bf16(
    C: int,
    repeat: int = 1,
    ph_a: int = 4,
    inner: str = "n",
    out_dt: str = "f32",
    lead: str = "orig",
    tail: str = "orig",
):
    """bf16 program for row capacity C.

    inner: "n"  — per (m, n) psum group, n inner (16-matmul groups)
           "ko" — per m-tile, 4 psum groups interleaved ko-outer (stationary
                  x[:, ko] shared by 4 consecutive matmuls; group-boundary
                  sync amortized 4x)
           "ko2"— per 2 m-tiles, 8 psum groups interleaved ko-outer
    out_dt: "f32" | "bf16" — HBM dtype of out (bf16 halves store traffic;
           host casts back, ~2e-3 extra rounding)
    lead: "fine" — 2-ko first x piece + 1-ko W chunks on the act queue so
          the first matmul starts ~1.8 us earlier; first phase-A wave is
          m={0} alone so it only waits on x tile 0
    tail: "narrow" — last m-tile ends with 256/128/128-wide psum groups so
          the final evict+DMA after the last matmul is ~3x shorter
    """
    key = ("bf16", C, repeat, ph_a, inner, out_dt, lead, tail)
    if key in _nc_cache:
        return _nc_cache[key]
    assert C % P == 0
    m_tiles = C // P
    f32 = mybir.dt.float32
    bf16 = mybir.dt.bfloat16
    odt = f32 if out_dt == "f32" else bf16

    nc = bacc.Bacc(
        "TRN2", target_bir_lowering=False, debug=False, num_devices=N_CORES
    )
    xT = nc.dram_tensor("xT", [m_tiles, P, KO, P], bf16, kind="ExternalInput").ap()
    wT = nc.dram_tensor("wT", [N_TILES, P, KO, N_TILE], bf16, kind="ExternalInput").ap()
    bb = nc.dram_tensor("bb", [P, D_OUT], f32, kind="ExternalInput").ap()
    out = nc.dram_tensor("out", [C, D_OUT], odt, kind="ExternalOutput").ap()

    ph_a = min(ph_a, m_tiles)

    with tile.TileContext(nc) as tc:
        with (
            tc.tile_pool(name="wpool", bufs=1) as wpool,
            tc.tile_pool(name="cpool", bufs=1) as cpool,
            tc.tile_pool(name="xapool", bufs=1) as xapool,
            tc.tile_pool(name="xpool", bufs=3) as xpool,
            tc.tile_pool(name="opool", bufs=4) as opool,
            tc.tile_pool(name="ofpool", bufs=4) as ofpool,
            tc.tile_pool(name="pspool", bufs=8, space="PSUM") as pspool,
        ):
            w_sb = wpool.tile([P, N_TILES, KO, N_TILE], bf16)
            b_sb = cpool.tile([P, D_OUT], f32)
            xa_sb = xapool.tile([P, ph_a, KO, P], bf16)

            KQ = 4
            n_kq = KO // KQ

            def w_chunk(n, kq):
                nc.sync.dma_start(
                    w_sb[:, n, kq * KQ : (kq + 1) * KQ],
                    wT[n, :, kq * KQ : (kq + 1) * KQ],
                )

            def xa_dma(m):
                nc.sync.dma_start(xa_sb[:, m], xT[m])

            if lead == "par":
                # n=0 W rides the act queue in 2/2/4/8-ko pieces, in parallel
                # with x on the sync queue: the first matmul waits only
                # max(x0h1, w0[0:2]) ~= 2 us, and the m={0}-only first wave
                # consumes slowly enough (p-state ramp) to track delivery.
                nc.sync.dma_start(xa_sb[:, 0, : KO // 2], xT[0, :, : KO // 2])
                nc.scalar.dma_start(w_sb[:, 0, :2], wT[0, :, :2])
                nc.scalar.dma_start(w_sb[:, 0, 2:4], wT[0, :, 2:4])
                nc.scalar.dma_start(w_sb[:, 0, 4:8], wT[0, :, 4:8])
                nc.scalar.dma_start(w_sb[:, 0, 8:], wT[0, :, 8:])
                nc.sync.dma_start(xa_sb[:, 0, KO // 2 :], xT[0, :, KO // 2 :])
                for m in range(1, ph_a):
                    xa_dma(m)
                nc.sync.dma_start(b_sb[:], bb[:])
                for n in range(1, N_TILES):
                    for kq in range(n_kq):
                        w_chunk(n, kq)
            elif lead == "fine":
                # x tile 0 in 2/6/8-ko pieces on the sync queue; n=0 W in
                # 1/1/2/4/8-ko pieces on the act queue (parallel DGE) paced
                # just ahead of the ko-loop's consumption. First matmul only
                # waits ~1.8 us for x0[0:2] + w0[0:1].
                nc.sync.dma_start(xa_sb[:, 0, :2], xT[0, :, :2])
                nc.scalar.dma_start(w_sb[:, 0, :1], wT[0, :, :1])
                nc.sync.dma_start(xa_sb[:, 0, 2:8], xT[0, :, 2:8])
                nc.scalar.dma_start(w_sb[:, 0, 1:2], wT[0, :, 1:2])
                nc.sync.dma_start(xa_sb[:, 0, 8:], xT[0, :, 8:])
                nc.scalar.dma_start(w_sb[:, 0, 2:4], wT[0, :, 2:4])
                nc.scalar.dma_start(w_sb[:, 0, 4:8], wT[0, :, 4:8])
                nc.scalar.dma_start(w_sb[:, 0, 8:], wT[0, :, 8:])
                for m in range(1, ph_a):
                    xa_dma(m)
                nc.sync.dma_start(b_sb[:], bb[:])
                for n in range(1, N_TILES):
                    for kq in range(n_kq):
                        w_chunk(n, kq)
            else:
                nc.sync.dma_start(xa_sb[:, 0, : KO // 2], xT[0, :, : KO // 2])
                nc.sync.dma_start(w_sb[:, 0, :2], wT[0, :, :2])
                nc.sync.dma_start(xa_sb[:, 0, KO // 2 :], xT[0, :, KO // 2 :])
                nc.sync.dma_start(w_sb[:, 0, 2:4], wT[0, :, 2:4])
                if ph_a > 1:
                    xa_dma(1)
                for kq in range(1, n_kq):
                    w_chunk(0, kq)
                for m in range(2, ph_a):
                    xa_dma(m)
                nc.sync.dma_start(b_sb[:], bb[:])
                for n in range(1, N_TILES):
                    for kq in range(n_kq):
                        w_chunk(n, kq)

            def evict(ps, m, n):
                ms = slice(m * P, (m + 1) * P)
                ns = slice(n * N_TILE, (n + 1) * N_TILE)
                o_sb = opool.tile([P, N_TILE], odt, tag="o", name=f"o_{m}_{n}")
                nc.vector.tensor_add(o_sb[:], ps, b_sb[:, ns])
                nc.scalar.dma_start(out[ms, ns], o_sb[:])

            def do_group(x_tile, m, n, o_full=None):
                ps = pspool.tile([P, N_TILE], f32, tag="ps")
                for ko in range(KO):
                    nc.tensor.matmul(
                        ps,
                        x_tile[:, ko],
                        w_sb[:, n, ko],
                        start=(ko == 0),
                        stop=(ko == KO - 1),
                    )
                if o_full is None:
                    evict(ps, m, n)
                else:
                    ns = slice(n * N_TILE, (n + 1) * N_TILE)
                    nc.vector.tensor_add(o_full[:, ns], ps, b_sb[:, ns])

            def do_group_cols(x_tile, m, c0, c1):
                """Narrow trailing psum group over out cols [c0, c1) (within
                one n-tile); psum/o_sb are slices of full-width pool tiles."""
                w = c1 - c0
                n = c0 // N_TILE
                ps_full = pspool.tile(
                    [P, N_TILE], f32, tag="ps", name=f"psn_{m}_{c0}"
                )
                ps = ps_full[:, :w]
                for ko in range(KO):
                    nc.tensor.matmul(
                        ps,
                        x_tile[:, ko],
                        w_sb[:, n, ko, c0 - n * N_TILE : c1 - n * N_TILE],
                        start=(ko == 0),
                        stop=(ko == KO - 1),
                    )
                o_sb = opool.tile(
                    [P, N_TILE], odt, tag="o", name=f"on_{m}_{c0}"
                )
                nc.vector.tensor_add(o_sb[:, :w], ps, b_sb[:, c0:c1])
                nc.scalar.dma_start(
                    out[m * P : (m + 1) * P, c0:c1], o_sb[:, :w]
                )

            def do_last_tile(x_tile, m):
                if tail == "narrow":
                    for n in range(N_TILES - 1):
                        do_group(x_tile, m, n)
                    base = (N_TILES - 1) * N_TILE
                    for c0, c1 in (
                        (base, base + 256),
                        (base + 256, base + 384),
                        (base + 384, base + 512),
                    ):
                        do_group_cols(x_tile, m, c0, c1)
                else:
                    for n in range(N_TILES):
                        do_group(x_tile, m, n)

            def do_mtiles_ko_outer(x_tiles, ms, o_fulls):
                """Interleave len(ms)*N_TILES psum groups ko-outer; 4
                consecutive matmuls share the stationary x[:, ko]."""
                pss = {}
                for m in ms:
                    for n in range(N_TILES):
                        pss[m, n] = pspool.tile(
                            [P, N_TILE], f32, tag="ps", name=f"ps_{m}_{n}"
                        )
                for ko in range(KO):
                    for mi, m in enumerate(ms):
                        for n in range(N_TILES):
                            nc.tensor.matmul(
                                pss[m, n],
                                x_tiles[mi][:, ko],
                                w_sb[:, n, ko],
                                start=(ko == 0),
                                stop=(ko == KO - 1),
                            )
                for mi, m in enumerate(ms):
                    for n in range(N_TILES):
                        ns = slice(n * N_TILE, (n + 1) * N_TILE)
                        if o_fulls[mi] is None:
                            evict(pss[m, n], m, n)
                        else:
                            nc.vector.tensor_add(
                                o_fulls[mi][:, ns], pss[m, n], b_sb[:, ns]
                            )

            def phase_a_quarter(n, m_set):
                pss = {}
                for m in m_set:
                    pss[m] = pspool.tile(
                        [P, N_TILE], f32, tag="ps", name=f"psA_{n}_{m}"
                    )
                for ko in range(KO):
                    for m in m_set:
                        nc.tensor.matmul(
                            pss[m],
                            xa_sb[:, m, ko],
                            w_sb[:, n, ko],
                            start=(ko == 0),
                            stop=(ko == KO - 1),
                        )
                for m in m_set:
                    evict(pss[m], m, n)

            for rep in range(repeat):
                if rep == 0:
                    n_first = 1 if lead in ("fine", "par") else min(2, ph_a)
                    first = list(range(n_first))
                    rest = list(range(n_first, ph_a))
                    phase_a_quarter(0, first)
                    if rest:
                        phase_a_quarter(0, rest)
                    for n in range(1, N_TILES):
                        phase_a_quarter(n, list(range(ph_a)))
                    b_start = ph_a
                else:
                    b_start = 0
                if inner == "n":
                    for m in range(b_start, m_tiles):
                        x_sb = xpool.tile([P, KO, P], bf16, tag="x")
                        nc.sync.dma_start(x_sb[:], xT[m])
                        if m == m_tiles - 1:
                            do_last_tile(x_sb, m)
                        else:
                            o_full = ofpool.tile([P, D_OUT], odt, tag="of")
                            for n in range(N_TILES):
                                do_group(x_sb, m, n, o_full=o_full)
                            nc.scalar.dma_start(
                                out[m * P : (m + 1) * P, :], o_full[:]
                            )
                else:
                    step = 2 if inner == "ko2" else 1
                    m = b_start
                    while m < m_tiles:
                        ms = list(range(m, min(m + step, m_tiles)))
                        x_tiles, o_fulls = [], []
                        for mi in ms:
                            x_sb = xpool.tile(
                                [P, KO, P], bf16, tag="x", name=f"x_{mi}"
                            )
                            nc.sync.dma_start(x_sb[:], xT[mi])
                            x_tiles.append(x_sb)
                            if mi == m_tiles - 1:
                                o_fulls.append(None)
                            else:
                                of_sb = ofpool.tile(
                                    [P, D_OUT], odt, tag="of", name=f"of_{mi}"
                                )
                                o_fulls.append(of_sb)
                        do_mtiles_ko_outer(x_tiles, ms, o_fulls)
                        for mi, of in zip(ms, o_fulls):
                            if of is not None:
                                nc.scalar.dma_start(
                                    out[mi * P : (mi + 1) * P, :], of[:]
                                )
                        m += step

    nc.compile()
    _nc_cache[key] = nc
    return nc


NB = 256  # borrow half-unit width (cols)


def build_program_bal(
    m_main: int, bh: int, rr=(), repeat: int = 1, ph_a: int = 4,
    lead2: bool = False, no_out: bool = False,
):
    """Balanced program: m_main own-expert m-tiles (full D_OUT) plus bh
    borrowed half-units (one x-tile x NB cols each, self-contained W/bias
    slices) redistributing overflow tiles of heavy experts. Perfectly
    balances total padded tile work across the 8 cores.
    """
    rr = tuple(rr)
    assert len(rr) == bh
    key = ("bal", m_main, bh, rr, repeat, ph_a, lead2, no_out)
    if key in _nc_cache:
        return _nc_cache[key]
    f32 = mybir.dt.float32
    bf16 = mybir.dt.bfloat16
    C = m_main * P

    nc = bacc.Bacc(
        "TRN2", target_bir_lowering=False, debug=False, num_devices=N_CORES
    )
    xT = nc.dram_tensor("xT", [m_main, P, KO, P], bf16, kind="ExternalInput").ap()
    wT = nc.dram_tensor("wT", [N_TILES, P, KO, N_TILE], bf16, kind="ExternalInput").ap()
    bb = nc.dram_tensor("bb", [P, D_OUT], f32, kind="ExternalInput").ap()
    xB = nc.dram_tensor("xB", [max(bh, 1), P, KO, P], bf16, kind="ExternalInput").ap()
    wB = nc.dram_tensor("wB", [max(bh, 1), P, KO, NB], bf16, kind="ExternalInput").ap()
    bB = nc.dram_tensor("bB", [P, max(bh, 1) * 2], f32, kind="ExternalInput").ap()
    out = nc.dram_tensor("out", [C, D_OUT], f32, kind="ExternalOutput").ap()
    # borrowed-unit outputs are TRANSPOSED: [unit, n-half, 128 n, r rows]
    outB = nc.dram_tensor(
        "outB", [max(bh, 1), 2, P, P], f32, kind="ExternalOutput"
    ).ap()

    ph_a = min(ph_a, m_main)

    with tile.TileContext(nc) as tc:
        with (
            tc.tile_pool(name="wpool", bufs=1) as wpool,
            tc.tile_pool(name="cpool", bufs=1) as cpool,
            tc.tile_pool(name="xapool", bufs=1) as xapool,
            tc.tile_pool(name="xpool", bufs=3) as xpool,
            tc.tile_pool(name="xbpool", bufs=max(bh, 1)) as xbpool,
            tc.tile_pool(name="opool", bufs=6) as opool,
            tc.tile_pool(name="ofpool", bufs=3) as ofpool,
            tc.tile_pool(name="pspool", bufs=8, space="PSUM") as pspool,
        ):
            w_sb = wpool.tile([P, N_TILES, KO, N_TILE], bf16, name="w")
            wb_sb = wpool.tile([P, max(bh, 1), KO, NB], bf16, name="wb")
            b_sb = cpool.tile([P, D_OUT], f32, name="b")
            bb_sb = cpool.tile([P, max(bh, 1) * 2], f32, name="bb2")
            xa_sb = xapool.tile([P, ph_a, KO, P], bf16, name="xa")

            KQ = 4
            n_kq = KO // KQ

            def w_chunk(n, kq):
                nc.sync.dma_start(
                    w_sb[:, n, kq * KQ : (kq + 1) * KQ],
                    wT[n, :, kq * KQ : (kq + 1) * KQ],
                )

            def xa_dma(m):
                nc.sync.dma_start(xa_sb[:, m], xT[m])

            wq = nc.scalar if lead2 else nc.sync
            nc.sync.dma_start(xa_sb[:, 0, : KO // 2], xT[0, :, : KO // 2])
            wq.dma_start(w_sb[:, 0, :2], wT[0, :, :2])
            nc.sync.dma_start(xa_sb[:, 0, KO // 2 :], xT[0, :, KO // 2 :])
            wq.dma_start(w_sb[:, 0, 2:4], wT[0, :, 2:4])
            if ph_a > 1:
                xa_dma(1)
            for kq in range(1, n_kq):
                (wq if lead2 else nc.sync).dma_start(
                    w_sb[:, 0, kq * KQ : (kq + 1) * KQ],
                    wT[0, :, kq * KQ : (kq + 1) * KQ],
                )
            for m in range(2, ph_a):
                xa_dma(m)
            nc.sync.dma_start(b_sb[:], bb[:])
            for n in range(1, N_TILES):
                for kq in range(n_kq):
                    w_chunk(n, kq)
            for u in range(bh):
                nc.sync.dma_start(wb_sb[:, u], wB[u])
            nc.sync.dma_start(bb_sb[:], bB[:])

            def evict(ps, m, n):
                if no_out:
                    return
                ms = slice(m * P, (m + 1) * P)
                ns = slice(n * N_TILE, (n + 1) * N_TILE)
                o_sb = opool.tile([P, N_TILE], f32, tag="o", name=f"o_{m}_{n}")
                nc.vector.tensor_add(o_sb[:], ps, b_sb[:, ns])
                nc.scalar.dma_start(out[ms, ns], o_sb[:])

            def do_group(x_tile, m, n, o_full=None):
                ps = pspool.tile([P, N_TILE], f32, tag="ps")
                for ko in range(KO):
                    nc.tensor.matmul(
                        ps,
                        x_tile[:, ko],
                        w_sb[:, n, ko],
                        start=(ko == 0),
                        stop=(ko == KO - 1),
                    )
                if no_out:
                    pass
                elif o_full is None:
                    evict(ps, m, n)
                else:
                    ns = slice(n * N_TILE, (n + 1) * N_TILE)
                    nc.vector.tensor_add(o_full[:, ns], ps, b_sb[:, ns])

            def do_group_cols(x_tile, m, c0, c1):
                w = c1 - c0
                n = c0 // N_TILE
                ps_full = pspool.tile(
                    [P, N_TILE], f32, tag="ps", name=f"psn_{m}_{c0}"
                )
                ps = ps_full[:, :w]
                for ko in range(KO):
                    nc.tensor.matmul(
                        ps,
                        x_tile[:, ko],
                        w_sb[:, n, ko, c0 - n * N_TILE : c1 - n * N_TILE],
                        start=(ko == 0),
                        stop=(ko == KO - 1),
                    )
                if no_out:
                    return
                o_sb = opool.tile(
                    [P, N_TILE], f32, tag="o", name=f"on_{m}_{c0}"
                )
                nc.vector.tensor_add(o_sb[:, :w], ps, b_sb[:, c0:c1])
                nc.scalar.dma_start(
                    out[m * P : (m + 1) * P, c0:c1], o_sb[:, :w]
                )

            def do_borrow_all():
                # rows are the MOVING dim: matmul cost ~ r cycles/col-tile,
                # so partial overflow tiles cost only their real rows.
                # Stationary = W col-slice [128k, 128n]; psum/out are
                # transposed ([n, rows]); bias is per-PARTITION (scalar AP).
                # All units interleave per n-half (<=8 psum banks) so the
                # group-entry sync of these tiny groups amortizes.
                live = [u for u in range(bh) if rr[u] > 0]
                if not live:
                    return
                xbs = {}
                for u in live:
                    x_sb = xbpool.tile(
                        [P, KO, P], bf16, tag="xb", name=f"xb_{u}"
                    )
                    nc.sync.dma_start(x_sb[:], xB[u])
                    xbs[u] = x_sb
                for nh in range(2):
                    pss = {}
                    for u in live:
                        pss[u] = pspool.tile(
                            [P, N_TILE], f32, tag="ps", name=f"psb_{u}_{nh}"
                        )
                    for ko in range(KO):
                        for u in live:
                            nc.tensor.matmul(
                                pss[u][:, : rr[u]],
                                wb_sb[:, u, ko, nh * P : (nh + 1) * P],
                                xbs[u][:, ko, : rr[u]],
                                start=(ko == 0),
                                stop=(ko == KO - 1),
                            )
                    if no_out:
                        continue
                    for u in live:
                        o_sb = opool.tile(
                            [P, N_TILE], f32, tag="o", name=f"ob_{u}_{nh}"
                        )
                        nc.vector.tensor_scalar_add(
                            o_sb[:, : rr[u]],
                            pss[u][:, : rr[u]],
                            bb_sb[:, 2 * u + nh : 2 * u + nh + 1],
                        )
                        nc.scalar.dma_start(
                            outB[u, nh, :, : rr[u]], o_sb[:, : rr[u]]
                        )

            def phase_a_quarter(n, m_set):
                pss = {}
                for m in m_set:
                    pss[m] = pspool.tile(
                        [P, N_TILE], f32, tag="ps", name=f"psA_{n}_{m}"
                    )
                for ko in range(KO):
                    for m in m_set:
                        nc.tensor.matmul(
                            pss[m],
                            xa_sb[:, m, ko],
                            w_sb[:, n, ko],
                            start=(ko == 0),
                            stop=(ko == KO - 1),
                        )
                for m in m_set:
                    evict(pss[m], m, n)

            for rep in range(repeat):
                if rep == 0:
                    first = list(range(min(2, ph_a)))
                    rest = list(range(min(2, ph_a), ph_a))
                    phase_a_quarter(0, first)
                    if rest:
                        phase_a_quarter(0, rest)
                    for n in range(1, N_TILES):
                        phase_a_quarter(n, list(range(ph_a)))
                    b_start = ph_a
                else:
                    b_start = 0
                # borrowed units run EARLY in each rep (but two tiles
                # in, so phase A's psum/DVE eviction backlog has drained):
                # their tiny evict+DMA chains (which would otherwise be a
                # ~9 us drain tail) hide under the main phase-B compute
                done_borrow = False
                for m in range(b_start, m_main):
                    if not done_borrow and m >= b_start + 2:
                        do_borrow_all()
                        done_borrow = True
                    x_sb = xpool.tile([P, KO, P], bf16, tag="x")
                    nc.sync.dma_start(x_sb[:], xT[m])
                    if no_out:
                        for n in range(N_TILES):
                            do_group(x_sb, m, n)
                        continue
                    if m == m_main - 1:
                        # last tile: per-slice eviction with a narrowing
                        # tail so the final evict+DMA after the last matmul
                        # is short
                        for n in range(N_TILES - 1):
                            do_group(x_sb, m, n)
                        base = (N_TILES - 1) * N_TILE
                        for c0, c1 in (
                            (base, base + 256),
                            (base + 256, base + 384),
                            (base + 384, base + 512),
                        ):
                            do_group_cols(x_sb, m, c0, c1)
                        continue
                    o_full = ofpool.tile([P, D_OUT], f32, tag="of")
                    for n in range(N_TILES):
                        do_group(x_sb, m, n, o_full=o_full)
                    nc.scalar.dma_start(out[m * P : (m + 1) * P, :], o_full[:])
                if not done_borrow:
                    do_borrow_all()

    nc.compile()
    _nc_cache[key] = nc
    return nc


def plan_bal(counts):
    """Balanced assignment. Each overflow tile (tiles beyond m_main of any
    expert) is split into 8 x NB-col units, one per core, so every core's
    borrow slots have IDENTICAL row counts rr (SPMD-uniform program) and
    the borrowed matmuls can use rows as the moving dim (cost ~ 32*r
    cycles instead of a full padded tile's 4096).
    Returns (m_main, bh, rr, units): units[c][u] = (g, mt, n2)."""
    tiles = [int(math.ceil(c / 128)) if c else 0 for c in counts]
    m_main = max(1, min(t for t in tiles))
    over = []  # (g, mt, r)
    for g in range(NUM_GROUPS):
        for mt in range(m_main, tiles[g]):
            over.append((g, mt, int(min(P, counts[g] - mt * P))))
    bh = len(over)
    rr = [r for (_g, _mt, r) in over]
    units = [
        [(g, mt, c) for (g, mt, _r) in over] for c in range(N_CORES)
    ]
    return m_main, bh, rr, units


BAL_PLAN = None


def shard_inputs(x, weight, bias, group_indices, mode: str | None = None):
    """Host-side expert-parallel sharding. Returns (in_maps, perm, offsets,
    counts, C)."""
    mode = MODE if mode is None else mode
    n_rows = x.shape[0]
    gi = np.asarray(group_indices)
    perm = np.argsort(gi, kind="stable")
    counts = np.bincount(gi, minlength=NUM_GROUPS).astype(np.int64)
    offsets = np.zeros(NUM_GROUPS + 1, dtype=np.int64)
    np.cumsum(counts, out=offsets[1:])
    C = max(P, int(math.ceil(counts.max() / P)) * P)

    x_sorted = x[perm] if not np.array_equal(perm, np.arange(n_rows)) else x
    m_tiles = C // P

    def block_x(xg):
        # [C, D_IN] -> [m, kp, ko, j] with xT[m, kp, ko, j] = xg[m*128+j, ko*128+kp]
        return np.ascontiguousarray(
            xg.reshape(m_tiles, P, KO, P).transpose(0, 3, 2, 1)
        )

    def block_w(wg):
        # W^T [D_IN, D_OUT] -> [n, kp, ko, nn]
        return np.ascontiguousarray(
            wg.T.reshape(KO, P, N_TILES, N_TILE).transpose(2, 1, 0, 3)
        )

    in_maps = []
    if mode == "bal":
        global BAL_PLAN
        m_main, bh, units = plan_bal(counts)
        BAL_PLAN = (m_main, bh, units)
        Cm = m_main * P

        def block_xg(xg, mt):
            return np.ascontiguousarray(
                xg.reshape(mt, P, KO, P).transpose(0, 3, 2, 1)
            ).astype(BF16)

        for c in range(N_CORES):
            g = c
            ng = int(min(counts[g], Cm))
            xg = np.zeros((Cm, D_IN), dtype=np.float32)
            xg[:ng] = x_sorted[offsets[g] : offsets[g] + ng]
            xBa = np.zeros((max(bh, 1), P, KO, P), dtype=BF16)
            wBa = np.zeros((max(bh, 1), P, KO, NB), dtype=BF16)
            bBa = np.zeros((P, max(bh, 1), NB), dtype=np.float32)
            for u, unit in enumerate(units[c]):
                if unit is None:
                    continue
                gu, mt, n2 = unit
                r0 = mt * P
                nr = int(min(P, counts[gu] - r0))
                xu = np.zeros((P, D_IN), dtype=np.float32)
                xu[:nr] = x_sorted[offsets[gu] + r0 : offsets[gu] + r0 + nr]
                xBa[u] = block_xg(xu, 1)[0]
                wslice = weight[gu].T[:, n2 * NB : (n2 + 1) * NB]
                wBa[u] = np.ascontiguousarray(
                    wslice.reshape(KO, P, NB).transpose(1, 0, 2)
                ).astype(BF16)
                bBa[:, u, :] = bias[gu][n2 * NB : (n2 + 1) * NB]
            in_maps.append(
                {
                    "xT": block_xg(xg, m_main),
                    "wT": block_w(weight[g]).astype(BF16),
                    "bb": np.ascontiguousarray(
                        np.broadcast_to(bias[g], (P, D_OUT))
                    ),
                    "xB": xBa,
                    "wB": wBa,
                    "bB": bBa,
                }
            )
        return in_maps, perm, offsets, counts, C
    if mode == "fp8":
        sx = np.float32(120.0) / max(np.abs(x).max(), 1e-30)
        sw = np.float32(120.0) / max(np.abs(weight).max(), 1e-30)
        c = np.float32(1.0) / (np.float32(sx) * np.float32(sw))
        cc = np.full((P, 1), c, dtype=np.float32)
        for g in range(NUM_GROUPS):
            ng = int(counts[g])
            xg = np.zeros((C, D_IN), dtype=np.float32)
            xg[:ng] = x_sorted[offsets[g] : offsets[g] + ng]
            xs = xg * sx
            x8 = xs.astype(F8)
            r8 = (xs - x8.astype(np.float32)).astype(F8)
            ws = weight[g] * sw
            w8 = ws.astype(F8)
            v8 = (ws - w8.astype(np.float32)).astype(F8)
            in_maps.append(
                {
                    "xT8": block_x(x8),
                    "xTr": block_x(r8),
                    "wT8": block_w(w8),
                    "wTv": block_w(v8),
                    "bb": np.ascontiguousarray(
                        np.broadcast_to(bias[g], (P, D_OUT))
                    ),
                    "cc": cc,
                }
            )
    else:
        for g in range(NUM_GROUPS):
            ng = int(counts[g])
            xg = np.zeros((C, D_IN), dtype=np.float32)
            xg[:ng] = x_sorted[offsets[g] : offsets[g] + ng]
            in_maps.append(
                {
                    "xT": block_x(xg).astype(BF16),
                    "wT": block_w(weight[g]).astype(BF16),
                    "bb": np.ascontiguousarray(
                        np.broadcast_to(bias[g], (P, D_OUT))
                    ),
                }
            )
    return in_maps, perm, offsets, counts, C


def unshard_output(results, perm, offsets, counts, n_rows):
    out = np.empty((n_rows, D_OUT), dtype=np.float32)
    for g in range(NUM_GROUPS):
        ng = int(counts[g])
        out[perm[offsets[g] : offsets[g] + ng]] = results[g]["out"][:ng]
    return out


def unshard_output_bal(results, perm, offsets, counts, n_rows):
    m_main, bh, units = BAL_PLAN
    Cm = m_main * P
    out = np.empty((n_rows, D_OUT), dtype=np.float32)
    for g in range(NUM_GROUPS):
        ng = int(min(counts[g], Cm))
        out[perm[offsets[g] : offsets[g] + ng]] = results[g]["out"][:ng]
    for c in range(N_CORES):
        for u, unit in enumerate(units[c]):
            if unit is None:
                continue
            gu, mt, n2 = unit
            r0 = mt * P
            nr = int(min(P, counts[gu] - r0))
            rows = perm[offsets[gu] + r0 : offsets[gu] + r0 + nr]
            out[rows, n2 * NB : (n2 + 1) * NB] = results[c]["outB"][
                u * P : u * P + nr
            ]
    return out


def kernel(x, weight, bias, group_indices):
    x = np.asarray(x, dtype=np.float32)
    weight = np.asarray(weight, dtype=np.float32)
    bias = np.asarray(bias, dtype=np.float32)
    group_indices = np.asarray(group_indices)
    assert x.shape[1] == D_IN and weight.shape == (NUM_GROUPS, D_OUT, D_IN)

    mode = MODE
    if mode == "bal":
        # fall back to the plain expert-parallel program when the group
        # distribution is too skewed for the borrow plan (keeps SBUF for
        # borrowed W slices bounded)
        gi = np.asarray(group_indices)
        cts = np.bincount(gi, minlength=NUM_GROUPS)
        m_main, bh, _units = plan_bal(cts)
        if bh > 8 or m_main < 4:
            mode = "bf16"
    in_maps, perm, offsets, counts, C = shard_inputs(
        x, weight, bias, group_indices, mode=mode
    )
    nc = build_program(C, mode=mode)
    res = run_bass_kernel_spmd(nc, in_maps, core_ids=list(range(N_CORES)))
    if mode == "bal":
        return unshard_output_bal(res.results, perm, offsets, counts, x.shape[0])
    return unshard_output(res.results, perm, offsets, counts, x.shape[0])


def _sim_main():
    import tsim

    C = 2176
    for mode in ("fp8", "bf16"):
        _nc_cache.clear()
        nc = build_program(C, repeat=1, mode=mode)
        t1, _ = tsim.simulate(nc)
        nc3 = build_program(C, repeat=3, mode=mode)
        t3, _ = tsim.simulate(nc3)
        body = (t3 - t1) / 2
        print(
            f"{mode}: full {t1 / 1e3:.1f} us, body {body / 1e3:.1f} us,"
            f" overhead {(t1 - body) / 1e3:.1f} us"
        )


if __name__ == "__main__":
    _sim_main()
